# revision 3
# baseline (speedup 1.0000x reference)
"""CBOW forward (embedding lookup -> ReLU -> vocab projection) on 8 TRN2 cores.

Full inputs in, full output out.  Sharding: every core computes the full
hidden layer h redundantly (the gather is tiny next to the output), and
the vocab axis of layer 2 is sharded 8 ways: core c computes
out[:, c*6250:(c+1)*6250] = h @ W2_c.T + b2_c.

Layer 1 (embedding sum): 8 independent indirect row-gathers per 128-row
block (one W1T row per partition per call — the only layout HW SWDGE
honors) land in an [n, 8, d] SBUF tile; a 3-level DVE tree sums them.
The gather table is bf16 with the 1/(2*CTX) scale pre-folded on the
host, halving gather bytes.  Duplicate context indices use scatter-SET
semantics (count once): the host redirects duplicate occurrences to an
appended all-zero row of W1T.  Two PE transposes per block produce hT,
with ReLU(x + b1) fused into the PSUM eviction on the Scalar engine.

Layer 2 runs a single bf16 term per K-half (fp32 PSUM accumulate):
~2e-3 scale-relative error, well inside the 2e-2 gate, at 1/3 the
TensorE time of the fp32-grade 3-term split.  The output is stored as
bf16 (halving the dominant write traffic) and upcast to fp32 on the
host.  PSUM->SBUF evictions are split between DVE and Scalar so
neither becomes the straggler.  When b2 is nonzero (not the case for
this problem's zero-filled biases) a variant with a DVE add against a
broadcast b2 row is compiled instead.  Emission is software-pipelined
(layer1 of block nb+1 ahead of layer2 of block nb).
"""

from contextlib import ExitStack

import numpy as np
import ml_dtypes

import concourse.bacc as bacc
import concourse.bass as bass
import concourse.mybir as mybir
import concourse.tile as tile
from concourse.masks import make_identity

# Problem shape (hardcoded per the task contract).
N = 2048          # batch
J = 8             # context window (2*CTX)
D = 256           # hidden
V = 50000         # vocab
C = 8             # cores
VS = V // C       # vocab shard per core = 6250
P = 128
NB = N // P       # 16 row-blocks
VT = 1024         # output tile width (two PSUM banks)
SCALAR_TILES = (2, 5)   # which VT-tiles the Scalar engine evicts

F32 = mybir.dt.float32
BF16 = mybir.dt.bfloat16
I32 = mybir.dt.int32

_CACHE = {}


def _build(with_b2):
    """Build + compile the single-core SPMD Bass program."""
    key = ("nc", with_b2)
    if key in _CACHE:
        return _CACHE[key]

    nc = bacc.Bacc("TRN2", target_bir_lowering=False, debug=False, num_devices=C)

    idx_d = nc.dram_tensor("idx", [P, NB * J], I32, kind="ExternalInput")
    w1t_d = nc.dram_tensor("w1t", [V + 1, D], BF16, kind="ExternalInput")
    w2t_d = nc.dram_tensor("w2t", [D, VS], BF16, kind="ExternalInput")
    b1_d = nc.dram_tensor("b1", [2, P, 1], F32, kind="ExternalInput")
    if with_b2:
        b2_d = nc.dram_tensor("b2", [1, VS], F32, kind="ExternalInput")
    out_d = nc.dram_tensor("out", [N, VS], BF16, kind="ExternalOutput")

    vsub = [(v, min(VT, VS - v)) for v in range(0, VS, VT)]  # 6x1024 + 106

    with tile.TileContext(nc) as tc, ExitStack() as ctx:
        const = ctx.enter_context(tc.tile_pool(name="const", bufs=1))
        gpool = ctx.enter_context(tc.tile_pool(name="g8", bufs=3))
        t4pool = ctx.enter_context(tc.tile_pool(name="t4", bufs=2))
        t2pool = ctx.enter_context(tc.tile_pool(name="t2", bufs=2))
        hpool = ctx.enter_context(tc.tile_pool(name="hraw", bufs=2))
        htpool = ctx.enter_context(tc.tile_pool(name="ht", bufs=4))
        opool = ctx.enter_context(tc.tile_pool(name="out", bufs=4))
        ps_s = ctx.enter_context(tc.tile_pool(name="ps_s", bufs=2, space="PSUM"))
        ps_b = ctx.enter_context(tc.tile_pool(name="ps_b", bufs=3, space="PSUM"))

        # ---- resident tensors -------------------------------------------
        idx_sb = const.tile([P, NB * J], I32, tag="idx")
        # block-0 indices first so its gathers start ~6us earlier
        nc.sync.dma_start(idx_sb[:, :J], idx_d[:, :J])
        nc.sync.dma_start(idx_sb[:, J:], idx_d[:, J:])
        ident = const.tile([P, P], F32, tag="ident")
        make_identity(nc, ident[:])
        b1t = [const.tile([P, 1], F32, tag=f"b1{h}", name=f"b1{h}")
               for h in (0, 1)]
        for h in (0, 1):
            nc.sync.dma_start(b1t[h][:], b1_d[h])
        w2 = [const.tile([P, VS], BF16, tag=f"w2{h}", name=f"w2{h}")
              for h in (0, 1)]
        for h in (0, 1):
            nc.sync.dma_start(w2[h][:], w2t_d[h * P:(h + 1) * P, :])
        if with_b2:
            # b2 broadcast to all 128 partitions via partition-step-0 DMA
            b2bc = const.tile([P, VS], F32, tag="b2bc")
            nc.sync.dma_start(b2bc[:], b2_d[:].to_broadcast([P, VS]))

        # ---- main pipeline ----------------------------------------------
        def stage_a(nb):
            # 8 independent slice-gathers (chained CCE-accumulate gathers
            # serialize the SWDGE queue at 2us/call; independent calls run
            # faster), then a 3-level DVE tree sum.
            g8 = gpool.tile([P, J, D], BF16, tag="g8", name="g8")
            for j in range(J):
                nc.gpsimd.indirect_dma_start(
                    out=g8[:, j, :],
                    out_offset=None,
                    in_=w1t_d[:],
                    in_offset=bass.IndirectOffsetOnAxis(
                        ap=idx_sb[:, nb * J + j:nb * J + j + 1], axis=0),
                )
            t4 = t4pool.tile([P, 4, D], BF16, tag="t4", name="t4")
            nc.vector.tensor_add(t4[:], g8[:, 0:4, :], g8[:, 4:8, :])
            t2 = t2pool.tile([P, 2, D], BF16, tag="t2", name="t2")
            nc.vector.tensor_add(t2[:], t4[:, 0:2, :], t4[:, 2:4, :])
            h_raw = hpool.tile([P, D], F32, tag="hraw", name="h_raw")
            nc.vector.tensor_add(h_raw[:], t2[:, 0, :], t2[:, 1, :])
            return h_raw

        def stage_b(nb, h_raw):
            # transpose to hT, fuse relu(x + b1), emit bf16.  Emitted AFTER
            # layer2(nb-1) so these PE transposes never head-of-line block
            # the previous block's matmuls in the PE queue.
            ht = [htpool.tile([P, P], BF16, tag=f"ht{h}", name=f"ht{h}")
                  for h in (0, 1)]
            for h in (0, 1):
                pt = ps_s.tile([P, 512], F32, tag="ps", name="pt")
                nc.tensor.transpose(pt[:, :P], h_raw[:, h * P:(h + 1) * P],
                                    ident[:])
                nc.scalar.activation(ht[h][:], pt[:, :P],
                                     mybir.ActivationFunctionType.Relu,
                                     bias=b1t[h][:], scale=1.0)
            return ht

        def layer2(nb, ht):
            # tile-pair groups with h as the OUTER loop: consecutive matmuls
            # share the same stationary lhsT, letting the weight load be
            # elided/overlapped instead of re-issued per matmul.
            groups = [(0, 2), (2, 4), (4, 6), (6, 7)]
            for g0, g1 in groups:
                pos = {}
                for i in range(g0, g1):
                    pos[i] = ps_b.tile([P, VT], F32, tag="po", name="po")
                for h in (0, 1):
                    for i in range(g0, g1):
                        v0, vw = vsub[i]
                        for sub in range(0, vw, 512):
                            sw = min(512, vw - sub)
                            nc.tensor.matmul(
                                pos[i][:, sub:sub + sw],
                                lhsT=ht[h][:],
                                rhs=w2[h][:, v0 + sub:v0 + sub + sw],
                                start=(h == 0),
                                stop=(h == 1))
                for i in range(g0, g1):
                    v0, vw = vsub[i]
                    ot = opool.tile([P, VT], BF16, tag="ot", name="ot")
                    if with_b2:
                        nc.vector.tensor_add(ot[:, :vw], pos[i][:, :vw],
                                             b2bc[:, v0:v0 + vw])
                    elif i % 2 == 1:
                        nc.scalar.copy(ot[:, :vw], pos[i][:, :vw])
                    else:
                        nc.vector.tensor_copy(ot[:, :vw], pos[i][:, :vw])
                    nc.sync.dma_start(out_d[nb * P:(nb + 1) * P, v0:v0 + vw],
                                      ot[:, :vw])

        # software pipeline: gathers+tree (stage A) run DEPTH_A blocks ahead
        # on gpsimd/DVE; transpose+relu (stage B) for nb+1 is emitted after
        # layer2(nb)'s matmuls so PE executes them between matmul batches.
        DEPTH_A = 2
        hraws = {nb: stage_a(nb) for nb in range(min(DEPTH_A, NB))}
        hts = {0: stage_b(0, hraws.pop(0))}
        for nb in range(NB):
            if nb + DEPTH_A < NB:
                hraws[nb + DEPTH_A] = stage_a(nb + DEPTH_A)
            layer2(nb, hts.pop(nb))
            if nb + 1 < NB:
                hts[nb + 1] = stage_b(nb + 1, hraws.pop(nb + 1))

    nc.compile()
    _CACHE[key] = nc
    return nc


def _host_prep(inputs, W1, b1, W2, b2):
    x = np.asarray(inputs)
    assert x.shape == (N, J) and x.dtype == np.int32

    # duplicate mask: scatter-SET semantics -> only first occurrence counts;
    # duplicates are redirected to the all-zero row V of the augmented W1T.
    dup = np.zeros((N, J), dtype=bool)
    for j in range(1, J):
        dup[:, j] = (x[:, :j] == x[:, j:j + 1]).any(axis=1)
    xd = np.where(dup, V, x).astype(np.int32)

    # idx2[p, nb*J + j] = xd[nb*128 + p, j]
    idx2 = np.ascontiguousarray(
        xd.reshape(NB, P, J).transpose(1, 0, 2).reshape(P, NB * J))

    w1 = np.asarray(W1, dtype=np.float32)
    w1t = np.concatenate([w1.T * (1.0 / J), np.zeros((1, D), np.float32)],
                         axis=0)
    w1t = np.ascontiguousarray(w1t).astype(ml_dtypes.bfloat16)   # [V+1, D]

    w2t = np.ascontiguousarray(np.asarray(W2, dtype=np.float32).T)  # [D, V]
    w2t = w2t.astype(ml_dtypes.bfloat16)

    b1r = np.ascontiguousarray(np.asarray(b1, dtype=np.float32).reshape(2, P, 1))
    b2f = np.asarray(b2, dtype=np.float32)
    with_b2 = bool(np.any(b2f))

    in_maps = []
    for c in range(C):
        sl = slice(c * VS, (c + 1) * VS)
        m = {
            "idx": idx2,
            "w1t": w1t,
            "w2t": np.ascontiguousarray(w2t[:, sl]),
            "b1": b1r,
        }
        if with_b2:
            m["b2"] = np.ascontiguousarray(b2f[sl].reshape(1, VS))
        in_maps.append(m)
    return in_maps, with_b2


def run(inputs, W1, b1, W2, b2, trace=False):
    from concourse.bass_utils import run_bass_kernel_spmd

    in_maps, with_b2 = _host_prep(inputs, W1, b1, W2, b2)
    nc = _build(with_b2)
    res = run_bass_kernel_spmd(nc, in_maps, core_ids=list(range(C)), trace=trace)
    out = np.concatenate(
        [np.asarray(res.results[c]["out"]) for c in range(C)], axis=1)
    return out.astype(np.float32), res


def kernel(inputs, W1, b1, W2, b2):
    out, _ = run(inputs, W1, b1, W2, b2, trace=False)
    return out


# revision 11
# speedup vs baseline: 1.0062x; 1.0062x over previous
"""CBOW forward (embedding lookup -> ReLU -> vocab projection) on 8 TRN2 cores.

Full inputs in, full output out.  Sharding: every core computes the full
hidden layer h redundantly (the gather is tiny next to the output), and
the vocab axis of layer 2 is sharded 8 ways: core c computes
out[:, c*6250:(c+1)*6250] = h @ W2_c.T + b2_c.

Layer 1 (embedding sum): 8 independent indirect row-gathers per 128-row
block (one W1T row per partition per call — the only layout HW SWDGE
honors) land in an [n, 8, d] SBUF tile; a 3-level DVE tree sums them.
The gather table is bf16 with the 1/(2*CTX) scale pre-folded on the
host, halving gather bytes.  Duplicate context indices use scatter-SET
semantics (count once): the host redirects duplicate occurrences to an
appended all-zero row of W1T.  Two PE transposes per block produce hT,
with ReLU(x + b1) fused into the PSUM eviction on the Scalar engine.

Layer 2 runs a single bf16 term per K-half (fp32 PSUM accumulate):
~2e-3 scale-relative error, well inside the 2e-2 gate, at 1/3 the
TensorE time of the fp32-grade 3-term split.  The output is stored as
bf16 (halving the dominant write traffic) and upcast to fp32 on the
host.  PSUM->SBUF evictions are split between DVE and Scalar so
neither becomes the straggler.  When b2 is nonzero (not the case for
this problem's zero-filled biases) a variant with a DVE add against a
broadcast b2 row is compiled instead.  Emission is software-pipelined
(layer1 of block nb+1 ahead of layer2 of block nb).
"""

from contextlib import ExitStack

import numpy as np
import ml_dtypes

import concourse.bacc as bacc
import concourse.bass as bass
import concourse.mybir as mybir
import concourse.tile as tile
from concourse.masks import make_identity

# Problem shape (hardcoded per the task contract).
N = 2048          # batch
J = 8             # context window (2*CTX)
D = 256           # hidden
V = 50000         # vocab
C = 8             # cores
VS = V // C       # vocab shard per core = 6250
P = 128
NB = N // P       # 16 row-blocks
VT = 1024         # output tile width (two PSUM banks)

F32 = mybir.dt.float32
BF16 = mybir.dt.bfloat16
I32 = mybir.dt.int32

_CACHE = {}


def _build(with_b2):
    """Build + compile the single-core SPMD Bass program."""
    key = ("nc", with_b2)
    if key in _CACHE:
        return _CACHE[key]

    nc = bacc.Bacc("TRN2", target_bir_lowering=False, debug=False, num_devices=C)

    idx_d = nc.dram_tensor("idx", [P, NB * J], I32, kind="ExternalInput")
    w1t_d = nc.dram_tensor("w1t", [V + 1, D], BF16, kind="ExternalInput")
    w2t_d = nc.dram_tensor("w2t", [D, VS], BF16, kind="ExternalInput")
    b1_d = nc.dram_tensor("b1", [2, P, 1], F32, kind="ExternalInput")
    if with_b2:
        b2_d = nc.dram_tensor("b2", [1, VS], F32, kind="ExternalInput")
    out_d = nc.dram_tensor("out", [N, VS], BF16, kind="ExternalOutput")

    vsub = [(v, min(VT, VS - v)) for v in range(0, VS, VT)]  # 6x1024 + 106

    with tile.TileContext(nc) as tc, ExitStack() as ctx:
        const = ctx.enter_context(tc.tile_pool(name="const", bufs=1))
        gpool = ctx.enter_context(tc.tile_pool(name="g8", bufs=4))
        t4pool = ctx.enter_context(tc.tile_pool(name="t4", bufs=2))
        t2pool = ctx.enter_context(tc.tile_pool(name="t2", bufs=2))
        hpool = ctx.enter_context(tc.tile_pool(name="hraw", bufs=2))
        htpool = ctx.enter_context(tc.tile_pool(name="ht", bufs=4))
        opool = ctx.enter_context(tc.tile_pool(name="out", bufs=4))
        ps_s = ctx.enter_context(tc.tile_pool(name="ps_s", bufs=2, space="PSUM"))
        ps_b = ctx.enter_context(tc.tile_pool(name="ps_b", bufs=3, space="PSUM"))

        # ---- resident tensors -------------------------------------------
        idx_sb = const.tile([P, NB * J], I32, tag="idx")
        # block-0 indices first so its gathers start ~6us earlier
        nc.sync.dma_start(idx_sb[:, :J], idx_d[:, :J])
        nc.sync.dma_start(idx_sb[:, J:], idx_d[:, J:])
        ident = const.tile([P, P], F32, tag="ident")
        make_identity(nc, ident[:])
        b1t = [const.tile([P, 1], F32, tag=f"b1{h}", name=f"b1{h}")
               for h in (0, 1)]
        for h in (0, 1):
            nc.sync.dma_start(b1t[h][:], b1_d[h])
        w2 = [const.tile([P, VS], BF16, tag=f"w2{h}", name=f"w2{h}")
              for h in (0, 1)]
        for h in (0, 1):
            nc.sync.dma_start(w2[h][:], w2t_d[h * P:(h + 1) * P, :])
        if with_b2:
            # b2 broadcast to all 128 partitions via partition-step-0 DMA
            b2bc = const.tile([P, VS], F32, tag="b2bc")
            nc.sync.dma_start(b2bc[:], b2_d[:].to_broadcast([P, VS]))

        # ---- main pipeline ----------------------------------------------
        def stage_a(nb):
            # indirect row-gathers only (gpsimd ops exclusively, so the
            # gather pipeline never waits on another engine).  GPAIR rows
            # per partition per call amortizes the ~1.4us/call SWDGE
            # issue+gap overhead.
            g8 = gpool.tile([P, J, D], BF16, tag="g8", name="g8")
            for j in range(J):
                nc.gpsimd.indirect_dma_start(
                    out=g8[:, j, :],
                    out_offset=None,
                    in_=w1t_d[:],
                    in_offset=bass.IndirectOffsetOnAxis(
                        ap=idx_sb[:, nb * J + j:nb * J + j + 1], axis=0),
                )
            return g8

        def stage_b(nb, g8):
            # 3-level DVE tree sum, transpose to hT on PE, then relu(x + b1)
            # fused into the PSUM eviction as one DVE tensor_scalar
            # (add b1, max 0) emitting bf16.
            t4 = t4pool.tile([P, 4, D], BF16, tag="t4", name="t4")
            nc.vector.tensor_add(t4[:], g8[:, 0:4, :], g8[:, 4:8, :])
            t2 = t2pool.tile([P, 2, D], BF16, tag="t2", name="t2")
            nc.vector.tensor_add(t2[:], t4[:, 0:2, :], t4[:, 2:4, :])
            h_raw = hpool.tile([P, D], F32, tag="hraw", name="h_raw")
            nc.vector.tensor_add(h_raw[:], t2[:, 0, :], t2[:, 1, :])
            ht = [htpool.tile([P, P], BF16, tag=f"ht{h}", name=f"ht{h}")
                  for h in (0, 1)]
            for h in (0, 1):
                pt = ps_s.tile([P, 512], F32, tag="ps", name="pt")
                nc.tensor.transpose(pt[:, :P], h_raw[:, h * P:(h + 1) * P],
                                    ident[:])
                nc.scalar.activation(ht[h][:], pt[:, :P],
                                     mybir.ActivationFunctionType.Relu,
                                     bias=b1t[h][:], scale=1.0)
            return ht

        def layer2_tiles(nb, ht, tiles):
            for i in tiles:
                v0, vw = vsub[i]
                po = ps_b.tile([P, VT], F32, tag="po", name="po")
                for sub in range(0, vw, 512):
                    sw = min(512, vw - sub)
                    for h in (0, 1):
                        nc.tensor.matmul(
                            po[:, sub:sub + sw],
                            lhsT=ht[h][:],
                            rhs=w2[h][:, v0 + sub:v0 + sub + sw],
                            start=(h == 0),
                            stop=(h == 1))
                ot = opool.tile([P, VT], BF16, tag="ot", name="ot")
                if with_b2:
                    nc.vector.tensor_add(ot[:, :vw], po[:, :vw],
                                         b2bc[:, v0:v0 + vw])
                elif i % 2 == 1:
                    nc.scalar.copy(ot[:, :vw], po[:, :vw])
                else:
                    nc.vector.tensor_copy(ot[:, :vw], po[:, :vw])
                nc.sync.dma_start(out_d[nb * P:(nb + 1) * P, v0:v0 + vw],
                                  ot[:, :vw])

        # software pipeline: gathers (stage A, gpsimd-only) run DEPTH_A
        # blocks ahead; tree+transpose+relu (stage B) for nb+1 is emitted
        # MID-layer2(nb): its DVE ops land between the two eviction halves
        # (by which time gathers nb+1 long finished) and its PE transposes
        # land mid-matmul-batch, so neither FIFO head-of-line blocks.
        DEPTH_A = 3
        g8s = {nb: stage_a(nb) for nb in range(min(DEPTH_A, NB))}
        hts = {0: stage_b(0, g8s.pop(0))}
        for nb in range(NB):
            if nb + DEPTH_A < NB:
                g8s[nb + DEPTH_A] = stage_a(nb + DEPTH_A)
            ht = hts.pop(nb)
            layer2_tiles(nb, ht, range(0, 3))
            if nb + 1 < NB:
                hts[nb + 1] = stage_b(nb + 1, g8s.pop(nb + 1))
            layer2_tiles(nb, ht, range(3, len(vsub)))

    nc.compile()
    _CACHE[key] = nc
    return nc


def _host_prep(inputs, W1, b1, W2, b2):
    x = np.asarray(inputs)
    assert x.shape == (N, J) and x.dtype == np.int32

    # duplicate mask: scatter-SET semantics -> only first occurrence counts;
    # duplicates are redirected to the all-zero row V of the augmented W1T.
    dup = np.zeros((N, J), dtype=bool)
    for j in range(1, J):
        dup[:, j] = (x[:, :j] == x[:, j:j + 1]).any(axis=1)
    xd = np.where(dup, V, x).astype(np.int32)

    # idx2[p, nb*J + j] = xd[nb*128 + p, j]
    idx2 = np.ascontiguousarray(
        xd.reshape(NB, P, J).transpose(1, 0, 2).reshape(P, NB * J))

    w1 = np.asarray(W1, dtype=np.float32)
    w1t = np.concatenate([w1.T * (1.0 / J), np.zeros((1, D), np.float32)],
                         axis=0)
    w1t = np.ascontiguousarray(w1t).astype(ml_dtypes.bfloat16)   # [V+1, D]

    w2t = np.ascontiguousarray(np.asarray(W2, dtype=np.float32).T)  # [D, V]
    w2t = w2t.astype(ml_dtypes.bfloat16)

    b1r = np.ascontiguousarray(np.asarray(b1, dtype=np.float32).reshape(2, P, 1))
    b2f = np.asarray(b2, dtype=np.float32)
    with_b2 = bool(np.any(b2f))

    in_maps = []
    for c in range(C):
        sl = slice(c * VS, (c + 1) * VS)
        m = {
            "idx": idx2,
            "w1t": w1t,
            "w2t": np.ascontiguousarray(w2t[:, sl]),
            "b1": b1r,
        }
        if with_b2:
            m["b2"] = np.ascontiguousarray(b2f[sl].reshape(1, VS))
        in_maps.append(m)
    return in_maps, with_b2


def run(inputs, W1, b1, W2, b2, trace=False):
    from concourse.bass_utils import run_bass_kernel_spmd

    in_maps, with_b2 = _host_prep(inputs, W1, b1, W2, b2)
    nc = _build(with_b2)
    res = run_bass_kernel_spmd(nc, in_maps, core_ids=list(range(C)), trace=trace)
    out = np.concatenate(
        [np.asarray(res.results[c]["out"]) for c in range(C)], axis=1)
    return out.astype(np.float32), res


def kernel(inputs, W1, b1, W2, b2):
    out, _ = run(inputs, W1, b1, W2, b2, trace=False)
    return out


# revision 12
# speedup vs baseline: 1.2792x; 1.2713x over previous
"""CBOW forward (embedding lookup -> ReLU -> vocab projection) on 8 TRN2 cores.

Full inputs in, full output out.  Sharding: data-parallel over the batch.
Core c owns row-blocks 2c and 2c+1 (256 examples) and computes the FULL
vocab projection for them: out[c*256:(c+1)*256, :] = relu(...) @ W2.T.

Rationale: the embedding gather is SWDGE descriptor-generation bound
(~8.5ns/descriptor, one gathered row per partition per call), so a
vocab-sharded layer 2 — which needs h for ALL 2048 examples on every
core — costs 128 serialized gather calls (~140us) per core, pacing the
whole kernel.  Batch sharding needs only 16 calls (~22us), finished
long before the matmul stream needs them.  The price is streaming all
of W2 (25.6MB bf16) through SBUF in double-buffered chunks, which at
~358GB/s hides comfortably under the ~180us TensorE stream.

Layer 1: 8 indirect row-gathers per 128-row block into an [n, 8, d]
bf16 SBUF tile (gather table pre-scaled by 1/(2*CTX) and bf16-cast on
host); 3-level DVE tree sum; PE transpose; ReLU(x + b1) on Scalar
fused into the PSUM eviction, emitting bf16 hT tiles.  Duplicate
context indices (scatter-SET semantics) are redirected on the host to
an appended all-zero row of W1T.

Layer 2 runs a single bf16 term per K-half (fp32 PSUM accumulate):
~5e-3 scale-relative error, well inside the 2e-2 gate, at 1/3 the
TensorE cost of an fp32-grade split.  The output is stored bf16
(halving the dominant write traffic) and upcast to fp32 on the host.
PSUM->SBUF evictions alternate between DVE and Scalar so neither
engine paces the PE.  When b2 is nonzero (not the case here: the
problem zero-fills both biases) a variant streaming broadcast b2
chunks with a DVE add is compiled instead.
"""

from contextlib import ExitStack

import numpy as np
import ml_dtypes

import concourse.bacc as bacc
import concourse.bass as bass
import concourse.mybir as mybir
import concourse.tile as tile
from concourse.masks import make_identity

# Problem shape (hardcoded per the task contract).
N = 2048          # batch
J = 8             # context window (2*CTX)
D = 256           # hidden
V = 50000         # vocab
C = 8             # cores
P = 128
KB = 2             # row-blocks per core (batch shard = 256 examples)
VT = 1024          # output tile width (two PSUM banks)
CHUNK = 4096       # W2 streaming chunk (columns)

F32 = mybir.dt.float32
BF16 = mybir.dt.bfloat16
I32 = mybir.dt.int32

_CACHE = {}


def _build(with_b2):
    """Build + compile the single-core SPMD Bass program."""
    key = ("nc", with_b2)
    if key in _CACHE:
        return _CACHE[key]

    nc = bacc.Bacc("TRN2", target_bir_lowering=False, debug=False, num_devices=C)

    idx_d = nc.dram_tensor("idx", [P, KB * J], I32, kind="ExternalInput")
    w1t_d = nc.dram_tensor("w1t", [V + 1, D], BF16, kind="ExternalInput")
    w2t_d = nc.dram_tensor("w2t", [D, V], BF16, kind="ExternalInput")
    b1_d = nc.dram_tensor("b1", [2, P, 1], F32, kind="ExternalInput")
    if with_b2:
        b2_d = nc.dram_tensor("b2", [1, V], F32, kind="ExternalInput")
    out_d = nc.dram_tensor("out", [KB * P, V], BF16, kind="ExternalOutput")

    chunks = [(c0, min(CHUNK, V - c0)) for c0 in range(0, V, CHUNK)]

    with tile.TileContext(nc) as tc, ExitStack() as ctx:
        const = ctx.enter_context(tc.tile_pool(name="const", bufs=1))
        gpool = ctx.enter_context(tc.tile_pool(name="g8", bufs=2))
        t4pool = ctx.enter_context(tc.tile_pool(name="t4", bufs=2))
        t2pool = ctx.enter_context(tc.tile_pool(name="t2", bufs=2))
        hpool = ctx.enter_context(tc.tile_pool(name="hraw", bufs=2))
        w2pool = ctx.enter_context(tc.tile_pool(name="w2c", bufs=4))
        opool = ctx.enter_context(tc.tile_pool(name="out", bufs=4))
        ps_s = ctx.enter_context(tc.tile_pool(name="ps_s", bufs=2, space="PSUM"))
        ps_b = ctx.enter_context(tc.tile_pool(name="ps_b", bufs=3, space="PSUM"))
        if with_b2:
            b2pool = ctx.enter_context(tc.tile_pool(name="b2c", bufs=2))

        # ---- resident tensors -------------------------------------------
        idx_sb = const.tile([P, KB * J], I32, tag="idx")
        nc.sync.dma_start(idx_sb[:], idx_d[:])
        ident = const.tile([P, P], F32, tag="ident")
        make_identity(nc, ident[:])
        b1t = [const.tile([P, 1], F32, tag=f"b1{h}", name=f"b1{h}")
               for h in (0, 1)]
        for h in (0, 1):
            nc.sync.dma_start(b1t[h][:], b1_d[h])
        # hT tiles for the core's KB blocks stay resident all kernel
        hts = [[const.tile([P, P], BF16, tag=f"ht{k}{h}", name=f"ht{k}{h}")
                for h in (0, 1)] for k in range(KB)]

        # ---- layer 1 (once; 2 blocks, 16 gather calls) ------------------
        g8s = []
        for k in range(KB):
            g8 = gpool.tile([P, J, D], BF16, tag="g8", name="g8")
            for j in range(J):
                nc.gpsimd.indirect_dma_start(
                    out=g8[:, j, :],
                    out_offset=None,
                    in_=w1t_d[:],
                    in_offset=bass.IndirectOffsetOnAxis(
                        ap=idx_sb[:, k * J + j:k * J + j + 1], axis=0),
                )
            g8s.append(g8)
        for k in range(KB):
            t4 = t4pool.tile([P, 4, D], BF16, tag="t4", name="t4")
            nc.vector.tensor_add(t4[:], g8s[k][:, 0:4, :], g8s[k][:, 4:8, :])
            t2 = t2pool.tile([P, 2, D], BF16, tag="t2", name="t2")
            nc.vector.tensor_add(t2[:], t4[:, 0:2, :], t4[:, 2:4, :])
            h_raw = hpool.tile([P, D], F32, tag="hraw", name="h_raw")
            nc.vector.tensor_add(h_raw[:], t2[:, 0, :], t2[:, 1, :])
            for h in (0, 1):
                pt = ps_s.tile([P, 512], F32, tag="ps", name="pt")
                nc.tensor.transpose(pt[:, :P], h_raw[:, h * P:(h + 1) * P],
                                    ident[:])
                nc.scalar.activation(hts[k][h][:], pt[:, :P],
                                     mybir.ActivationFunctionType.Relu,
                                     bias=b1t[h][:], scale=1.0)

        # ---- layer 2: stream W2 chunks, both blocks per chunk -----------
        def load_chunk(ci):
            c0, cw = chunks[ci]
            w2c = [w2pool.tile([P, CHUNK], BF16, tag=f"w2c{h}", name=f"w2c{h}")
                   for h in (0, 1)]
            for h in (0, 1):
                nc.sync.dma_start(w2c[h][:, :cw], w2t_d[h * P:(h + 1) * P,
                                                        c0:c0 + cw])
            if with_b2:
                b2c = b2pool.tile([P, CHUNK], F32, tag="b2c", name="b2c")
                nc.sync.dma_start(b2c[:, :cw],
                                  b2_d[:, c0:c0 + cw].to_broadcast([P, cw]))
                return w2c, b2c
            return w2c, None

        DEPTH = 2
        pend = {ci: load_chunk(ci) for ci in range(min(DEPTH, len(chunks)))}
        ei = 0  # eviction round-robin
        for ci, (c0, cw) in enumerate(chunks):
            if ci + DEPTH < len(chunks):
                pend[ci + DEPTH] = load_chunk(ci + DEPTH)
            w2c, b2c = pend.pop(ci)
            for k in range(KB):
                for v0 in range(0, cw, VT):
                    vw = min(VT, cw - v0)
                    po = ps_b.tile([P, VT], F32, tag="po", name="po")
                    for sub in range(0, vw, 512):
                        sw = min(512, vw - sub)
                        for h in (0, 1):
                            nc.tensor.matmul(
                                po[:, sub:sub + sw],
                                lhsT=hts[k][h][:],
                                rhs=w2c[h][:, v0 + sub:v0 + sub + sw],
                                start=(h == 0),
                                stop=(h == 1))
                    ot = opool.tile([P, VT], BF16, tag="ot", name="ot")
                    if with_b2:
                        nc.vector.tensor_add(ot[:, :vw], po[:, :vw],
                                             b2c[:, v0:v0 + vw])
                    elif ei % 2 == 1:
                        nc.scalar.copy(ot[:, :vw], po[:, :vw])
                    else:
                        nc.vector.tensor_copy(ot[:, :vw], po[:, :vw])
                    ei += 1
                    nc.sync.dma_start(
                        out_d[k * P:(k + 1) * P, c0 + v0:c0 + v0 + vw],
                        ot[:, :vw])

    nc.compile()
    _CACHE[key] = nc
    return nc


def _host_prep(inputs, W1, b1, W2, b2):
    x = np.asarray(inputs)
    assert x.shape == (N, J) and x.dtype == np.int32

    # duplicate mask: scatter-SET semantics -> only first occurrence counts;
    # duplicates are redirected to the all-zero row V of the augmented W1T.
    dup = np.zeros((N, J), dtype=bool)
    for j in range(1, J):
        dup[:, j] = (x[:, :j] == x[:, j:j + 1]).any(axis=1)
    xd = np.where(dup, V, x).astype(np.int32)

    # idx2[c][p, k*J + j] = xd[(2c+k)*128 + p, j]
    idx2 = np.ascontiguousarray(
        xd.reshape(C, KB, P, J).transpose(0, 2, 1, 3).reshape(C, P, KB * J))

    w1 = np.asarray(W1, dtype=np.float32)
    w1t = np.concatenate([w1.T * (1.0 / J), np.zeros((1, D), np.float32)],
                         axis=0)
    w1t = np.ascontiguousarray(w1t).astype(ml_dtypes.bfloat16)   # [V+1, D]

    w2t = np.ascontiguousarray(np.asarray(W2, dtype=np.float32).T)  # [D, V]
    w2t = w2t.astype(ml_dtypes.bfloat16)

    b1r = np.ascontiguousarray(np.asarray(b1, dtype=np.float32).reshape(2, P, 1))
    b2f = np.asarray(b2, dtype=np.float32)
    with_b2 = bool(np.any(b2f))

    in_maps = []
    for c in range(C):
        m = {
            "idx": idx2[c],
            "w1t": w1t,
            "w2t": w2t,
            "b1": b1r,
        }
        if with_b2:
            m["b2"] = np.ascontiguousarray(b2f.reshape(1, V))
        in_maps.append(m)
    return in_maps, with_b2


def run(inputs, W1, b1, W2, b2, trace=False):
    from concourse.bass_utils import run_bass_kernel_spmd

    in_maps, with_b2 = _host_prep(inputs, W1, b1, W2, b2)
    nc = _build(with_b2)
    res = run_bass_kernel_spmd(nc, in_maps, core_ids=list(range(C)), trace=trace)
    out = np.concatenate(
        [np.asarray(res.results[c]["out"]) for c in range(C)], axis=0)
    return out.astype(np.float32), res


def kernel(inputs, W1, b1, W2, b2):
    out, _ = run(inputs, W1, b1, W2, b2, trace=False)
    return out


# revision 15
# speedup vs baseline: 1.3998x; 1.0943x over previous
"""CBOW forward (embedding lookup -> ReLU -> vocab projection) on 8 TRN2 cores.

Full inputs in, full output out.  Sharding: data-parallel over the batch.
Core c owns row-blocks 2c and 2c+1 (256 examples) and computes the FULL
vocab projection for them: out[c*256:(c+1)*256, :] = relu(...) @ W2.T.

Rationale: the embedding gather is SWDGE descriptor-generation bound
(~8.5ns/descriptor, one gathered row per partition per call), so a
vocab-sharded layer 2 — which needs h for ALL 2048 examples on every
core — costs 128 serialized gather calls (~140us) per core, pacing the
whole kernel.  Batch sharding needs only 16 calls (~22us), finished
long before the matmul stream needs them.  The price is streaming all
of W2 (25.6MB bf16) through SBUF in double-buffered chunks, which at
~358GB/s hides comfortably under the ~180us TensorE stream.

Layer 1: 8 indirect row-gathers per 128-row block into an [n, 8, d]
bf16 SBUF tile (gather table pre-scaled by 1/(2*CTX) and bf16-cast on
host); 3-level DVE tree sum; PE transpose; ReLU(x + b1) on Scalar
fused into the PSUM eviction, emitting bf16 hT tiles.  Duplicate
context indices (scatter-SET semantics) are redirected on the host to
an appended all-zero row of W1T.

Layer 2 runs a single bf16 term per K-half (fp32 PSUM accumulate):
~5e-3 scale-relative error, well inside the 2e-2 gate, at 1/3 the
TensorE cost of an fp32-grade split.  The output is stored bf16
(halving the dominant write traffic) and upcast to fp32 on the host.
PSUM->SBUF evictions alternate between DVE and Scalar so neither
engine paces the PE.  When b2 is nonzero (not the case here: the
problem zero-fills both biases) a variant streaming broadcast b2
chunks with a DVE add is compiled instead.
"""

from contextlib import ExitStack

import numpy as np
import ml_dtypes

import concourse.bacc as bacc
import concourse.bass as bass
import concourse.mybir as mybir
import concourse.tile as tile
from concourse.masks import make_identity

# Problem shape (hardcoded per the task contract).
N = 2048          # batch
J = 8             # context window (2*CTX)
D = 256           # hidden
V = 50000         # vocab
C = 8             # cores
P = 128
KB = 2             # row-blocks per core (batch shard = 256 examples)
VT = 1024          # output tile width (two PSUM banks)
CHUNK = 4096       # W2 streaming chunk (columns)

F32 = mybir.dt.float32
BF16 = mybir.dt.bfloat16
I32 = mybir.dt.int32

_CACHE = {}


def _build(with_b2):
    """Build + compile the single-core SPMD Bass program."""
    key = ("nc", with_b2)
    if key in _CACHE:
        return _CACHE[key]

    nc = bacc.Bacc("TRN2", target_bir_lowering=False, debug=False, num_devices=C)

    idx_d = nc.dram_tensor("idx", [P, KB * J], I32, kind="ExternalInput")
    w1t_d = nc.dram_tensor("w1t", [V + 1, D], BF16, kind="ExternalInput")
    w2t_d = nc.dram_tensor("w2t", [D, V], BF16, kind="ExternalInput")
    b1_d = nc.dram_tensor("b1", [2, P, 1], F32, kind="ExternalInput")
    if with_b2:
        b2_d = nc.dram_tensor("b2", [1, V], F32, kind="ExternalInput")
    out_d = nc.dram_tensor("out", [KB * P, V], BF16, kind="ExternalOutput")

    chunks = [(c0, min(CHUNK, V - c0)) for c0 in range(0, V, CHUNK)]

    with tile.TileContext(nc) as tc, ExitStack() as ctx:
        const = ctx.enter_context(tc.tile_pool(name="const", bufs=1))
        gpool = ctx.enter_context(tc.tile_pool(name="g8", bufs=2))
        t4pool = ctx.enter_context(tc.tile_pool(name="t4", bufs=2))
        t2pool = ctx.enter_context(tc.tile_pool(name="t2", bufs=2))
        hpool = ctx.enter_context(tc.tile_pool(name="hraw", bufs=2))
        w2pool = ctx.enter_context(tc.tile_pool(name="w2c", bufs=6))
        opool = ctx.enter_context(tc.tile_pool(name="out", bufs=4))
        ps_s = ctx.enter_context(tc.tile_pool(name="ps_s", bufs=2, space="PSUM"))
        ps_b = ctx.enter_context(tc.tile_pool(name="ps_b", bufs=3, space="PSUM"))
        if with_b2:
            b2pool = ctx.enter_context(tc.tile_pool(name="b2c", bufs=2))

        # ---- resident tensors -------------------------------------------
        idx_sb = const.tile([P, KB * J], I32, tag="idx")
        nc.sync.dma_start(idx_sb[:], idx_d[:])
        ident = const.tile([P, P], F32, tag="ident")
        make_identity(nc, ident[:])
        b1t = [const.tile([P, 1], F32, tag=f"b1{h}", name=f"b1{h}")
               for h in (0, 1)]
        for h in (0, 1):
            nc.sync.dma_start(b1t[h][:], b1_d[h])
        # hT tiles for the core's KB blocks stay resident all kernel
        hts = [[const.tile([P, P], BF16, tag=f"ht{k}{h}", name=f"ht{k}{h}")
                for h in (0, 1)] for k in range(KB)]

        # ---- layer 1 (once; 2 blocks, 16 gather calls) ------------------
        def gather_block(k):
            g8 = gpool.tile([P, J, D], BF16, tag="g8", name="g8")
            for j in range(J):
                nc.gpsimd.indirect_dma_start(
                    out=g8[:, j, :],
                    out_offset=None,
                    in_=w1t_d[:],
                    in_offset=bass.IndirectOffsetOnAxis(
                        ap=idx_sb[:, k * J + j:k * J + j + 1], axis=0),
                )
            return g8

        def finish_block(k, g8):
            t4 = t4pool.tile([P, 4, D], BF16, tag="t4", name="t4")
            nc.vector.tensor_add(t4[:], g8[:, 0:4, :], g8[:, 4:8, :])
            t2 = t2pool.tile([P, 2, D], BF16, tag="t2", name="t2")
            nc.vector.tensor_add(t2[:], t4[:, 0:2, :], t4[:, 2:4, :])
            h_raw = hpool.tile([P, D], F32, tag="hraw", name="h_raw")
            nc.vector.tensor_add(h_raw[:], t2[:, 0, :], t2[:, 1, :])
            for h in (0, 1):
                pt = ps_s.tile([P, 512], F32, tag="ps", name="pt")
                nc.tensor.transpose(pt[:, :P], h_raw[:, h * P:(h + 1) * P],
                                    ident[:])
                nc.scalar.activation(hts[k][h][:], pt[:, :P],
                                     mybir.ActivationFunctionType.Relu,
                                     bias=b1t[h][:], scale=1.0)

        g8s = [gather_block(k) for k in range(KB)]
        # block 0's tree/transpose/relu emitted now; block 1's is deferred
        # into the middle of chunk 0's matmuls (see below) so its PE
        # transposes don't head-of-line block chunk 0 / block 0 matmuls
        # while block 1's gathers are still in flight.
        finish_block(0, g8s[0])

        # ---- layer 2: stream W2 chunks, both blocks per chunk -----------
        def load_chunk(ci):
            c0, cw = chunks[ci]
            w2c = [w2pool.tile([P, CHUNK], BF16, tag=f"w2c{h}", name=f"w2c{h}")
                   for h in (0, 1)]
            for h in (0, 1):
                nc.sync.dma_start(w2c[h][:, :cw], w2t_d[h * P:(h + 1) * P,
                                                        c0:c0 + cw])
            if with_b2:
                b2c = b2pool.tile([P, CHUNK], F32, tag="b2c", name="b2c")
                nc.sync.dma_start(b2c[:, :cw],
                                  b2_d[:, c0:c0 + cw].to_broadcast([P, cw]))
                return w2c, b2c
            return w2c, None

        DEPTH = 3
        pend = {ci: load_chunk(ci) for ci in range(min(DEPTH, len(chunks)))}
        ei = 0  # eviction round-robin
        for ci, (c0, cw) in enumerate(chunks):
            if ci + DEPTH < len(chunks):
                pend[ci + DEPTH] = load_chunk(ci + DEPTH)
            w2c, b2c = pend.pop(ci)
            for k in range(KB):
                if ci == 0 and k == 1:
                    finish_block(1, g8s[1])
                for v0 in range(0, cw, VT):
                    vw = min(VT, cw - v0)
                    po = ps_b.tile([P, VT], F32, tag="po", name="po")
                    for sub in range(0, vw, 512):
                        sw = min(512, vw - sub)
                        for h in (0, 1):
                            nc.tensor.matmul(
                                po[:, sub:sub + sw],
                                lhsT=hts[k][h][:],
                                rhs=w2c[h][:, v0 + sub:v0 + sub + sw],
                                start=(h == 0),
                                stop=(h == 1))
                    ot = opool.tile([P, VT], BF16, tag="ot", name="ot")
                    if with_b2:
                        nc.vector.tensor_add(ot[:, :vw], po[:, :vw],
                                             b2c[:, v0:v0 + vw])
                    elif ei % 2 == 1:
                        nc.scalar.copy(ot[:, :vw], po[:, :vw])
                    else:
                        nc.vector.tensor_copy(ot[:, :vw], po[:, :vw])
                    ei += 1
                    nc.sync.dma_start(
                        out_d[k * P:(k + 1) * P, c0 + v0:c0 + v0 + vw],
                        ot[:, :vw])

    nc.compile()
    _CACHE[key] = nc
    return nc


def _host_prep(inputs, W1, b1, W2, b2):
    x = np.asarray(inputs)
    assert x.shape == (N, J) and x.dtype == np.int32

    # duplicate mask: scatter-SET semantics -> only first occurrence counts;
    # duplicates are redirected to the all-zero row V of the augmented W1T.
    dup = np.zeros((N, J), dtype=bool)
    for j in range(1, J):
        dup[:, j] = (x[:, :j] == x[:, j:j + 1]).any(axis=1)
    xd = np.where(dup, V, x).astype(np.int32)

    # idx2[c][p, k*J + j] = xd[(2c+k)*128 + p, j]
    idx2 = np.ascontiguousarray(
        xd.reshape(C, KB, P, J).transpose(0, 2, 1, 3).reshape(C, P, KB * J))

    w1 = np.asarray(W1, dtype=np.float32)
    w1t = np.concatenate([w1.T * (1.0 / J), np.zeros((1, D), np.float32)],
                         axis=0)
    w1t = np.ascontiguousarray(w1t).astype(ml_dtypes.bfloat16)   # [V+1, D]

    w2t = np.ascontiguousarray(np.asarray(W2, dtype=np.float32).T)  # [D, V]
    w2t = w2t.astype(ml_dtypes.bfloat16)

    b1r = np.ascontiguousarray(np.asarray(b1, dtype=np.float32).reshape(2, P, 1))
    b2f = np.asarray(b2, dtype=np.float32)
    with_b2 = bool(np.any(b2f))

    in_maps = []
    for c in range(C):
        m = {
            "idx": idx2[c],
            "w1t": w1t,
            "w2t": w2t,
            "b1": b1r,
        }
        if with_b2:
            m["b2"] = np.ascontiguousarray(b2f.reshape(1, V))
        in_maps.append(m)
    return in_maps, with_b2


def run(inputs, W1, b1, W2, b2, trace=False):
    from concourse.bass_utils import run_bass_kernel_spmd

    in_maps, with_b2 = _host_prep(inputs, W1, b1, W2, b2)
    nc = _build(with_b2)
    res = run_bass_kernel_spmd(nc, in_maps, core_ids=list(range(C)), trace=trace)
    out = np.concatenate(
        [np.asarray(res.results[c]["out"]) for c in range(C)], axis=0)
    return out.astype(np.float32), res


def kernel(inputs, W1, b1, W2, b2):
    out, _ = run(inputs, W1, b1, W2, b2, trace=False)
    return out


# revision 17
# speedup vs baseline: 1.4515x; 1.0369x over previous
"""CBOW forward (embedding lookup -> ReLU -> vocab projection) on 8 TRN2 cores.

Full inputs in, full output out.  Sharding: 2 vocab halves x 4 batch
quarters.  Core c owns batch quarter Q = c % 4 (blocks 4Q..4Q+3, 512
examples) and vocab half g = c // 4 (columns [25000*g, 25000*(g+1))):
out[512*Q:512*(Q+1), 25000*g:25000*(g+1)] = relu(...) @ W2_g.T.

Why this split: the embedding gather is SWDGE descriptor-generation
bound (~1.4us per 128-row call, one gathered row per partition), so
per-core gather cost scales with the batch shard: a full-batch
(vocab-only-sharded) core needs 128 calls (~180us) and paces the
kernel; a quarter-batch core needs 32 (~45us), finished far ahead of
the matmul stream.  Meanwhile the vocab half keeps W2 resident in SBUF
(2 x [128, 25000] bf16 = ~98KB/partition), so the steady state moves
only the output (25.6MB bf16/core) - well under the ~366GB/s DMA
subsystem - and the TensorEngine stream (~2 x 200 matmul-512s at
~216ns warm cadence) becomes the binding resource.  The W2 load is
split into column segments so the first matmuls only wait on their own
segment.

Layer 1: 8 indirect row-gathers per 128-row block into an [n, 8, d]
bf16 SBUF tile (gather table bf16, pre-scaled by 1/(2*CTX) on host);
3-level DVE tree sum; PE transpose; ReLU(x + b1) on Scalar fused into
the PSUM eviction, emitting resident bf16 hT tiles.  Duplicate context
indices (scatter-SET semantics) are redirected on the host to an
appended all-zero row of W1T.

Layer 2 runs a single bf16 term per K-half (fp32 PSUM accumulate):
~5e-3 scale-relative error, inside the 2e-2 gate, at 1/3 the TensorE
cost of an fp32-grade split.  Output is stored bf16 (halving the
dominant write) and upcast to fp32 on host.  PSUM->SBUF evictions
round-robin over DVE+Scalar (+GpSimd once its gathers are done) so no
single engine paces the PE.  When b2 is nonzero (not the case here:
the problem zero-fills both biases) a variant with a resident bf16
broadcast b2 and DVE adds is compiled instead.
"""

from contextlib import ExitStack

import numpy as np
import ml_dtypes

import concourse.bacc as bacc
import concourse.bass as bass
import concourse.mybir as mybir
import concourse.tile as tile
from concourse.masks import make_identity

# Problem shape (hardcoded per the task contract).
N = 2048          # batch
J = 8             # context window (2*CTX)
D = 256           # hidden
V = 50000         # vocab
C = 8             # cores
P = 128
GV = 2             # vocab groups
KB = 4             # row-blocks per core (batch quarter = 512 examples)
VS = V // GV       # vocab columns per core = 25000
VT = 1024          # output tile width (two PSUM banks)
WSEG = 3125        # W2 resident-load column segment

F32 = mybir.dt.float32
BF16 = mybir.dt.bfloat16
I32 = mybir.dt.int32

_CACHE = {}


def _build(with_b2):
    """Build + compile the single-core SPMD Bass program."""
    key = ("nc", with_b2)
    if key in _CACHE:
        return _CACHE[key]

    nc = bacc.Bacc("TRN2", target_bir_lowering=False, debug=False, num_devices=C)

    idx_d = nc.dram_tensor("idx", [P, KB * J], I32, kind="ExternalInput")
    w1t_d = nc.dram_tensor("w1t", [V + 1, D], BF16, kind="ExternalInput")
    w2t_d = nc.dram_tensor("w2t", [D, VS], BF16, kind="ExternalInput")
    b1_d = nc.dram_tensor("b1", [2, P, 1], F32, kind="ExternalInput")
    if with_b2:
        b2_d = nc.dram_tensor("b2", [1, VS], F32, kind="ExternalInput")
    out_d = nc.dram_tensor("out", [KB * P, VS], BF16, kind="ExternalOutput")

    with tile.TileContext(nc) as tc, ExitStack() as ctx:
        const = ctx.enter_context(tc.tile_pool(name="const", bufs=1))
        gpool = ctx.enter_context(tc.tile_pool(name="g8", bufs=3))
        t4pool = ctx.enter_context(tc.tile_pool(name="t4", bufs=2))
        t2pool = ctx.enter_context(tc.tile_pool(name="t2", bufs=2))
        hpool = ctx.enter_context(tc.tile_pool(name="hraw", bufs=2))
        opool = ctx.enter_context(tc.tile_pool(name="out", bufs=6))
        ps_s = ctx.enter_context(tc.tile_pool(name="ps_s", bufs=2, space="PSUM"))
        ps_b = ctx.enter_context(tc.tile_pool(name="ps_b", bufs=3, space="PSUM"))

        # ---- resident tensors -------------------------------------------
        idx_sb = const.tile([P, KB * J], I32, tag="idx")
        nc.sync.dma_start(idx_sb[:], idx_d[:])
        ident = const.tile([P, P], F32, tag="ident")
        make_identity(nc, ident[:])
        b1t = [const.tile([P, 1], F32, tag=f"b1{h}", name=f"b1{h}")
               for h in (0, 1)]
        for h in (0, 1):
            nc.sync.dma_start(b1t[h][:], b1_d[h])
        # resident W2 half, loaded in column segments (early matmuls only
        # wait on their own segment thanks to subtile dependency tracking)
        w2r = [const.tile([P, VS], BF16, tag=f"w2r{h}", name=f"w2r{h}")
               for h in (0, 1)]
        for s0 in range(0, VS, WSEG):
            sw = min(WSEG, VS - s0)
            for h in (0, 1):
                nc.sync.dma_start(w2r[h][:, s0:s0 + sw],
                                  w2t_d[h * P:(h + 1) * P, s0:s0 + sw])
        if with_b2:
            b2bc = const.tile([P, VS], BF16, tag="b2bc")
            nc.sync.dma_start(b2bc[:], b2_d[:].to_broadcast([P, VS]))
        # resident hT tiles for the core's KB blocks
        hts = [[const.tile([P, P], BF16, tag=f"ht{k}{h}", name=f"ht{k}{h}")
                for h in (0, 1)] for k in range(KB)]

        # ---- layer 1 ----------------------------------------------------
        def gather_block(k):
            g8 = gpool.tile([P, J, D], BF16, tag="g8", name="g8")
            for j in range(J):
                nc.gpsimd.indirect_dma_start(
                    out=g8[:, j, :],
                    out_offset=None,
                    in_=w1t_d[:],
                    in_offset=bass.IndirectOffsetOnAxis(
                        ap=idx_sb[:, k * J + j:k * J + j + 1], axis=0),
                )
            return g8

        def finish_block(k, g8):
            t4 = t4pool.tile([P, 4, D], BF16, tag="t4", name="t4")
            nc.vector.tensor_add(t4[:], g8[:, 0:4, :], g8[:, 4:8, :])
            t2 = t2pool.tile([P, 2, D], BF16, tag="t2", name="t2")
            nc.vector.tensor_add(t2[:], t4[:, 0:2, :], t4[:, 2:4, :])
            h_raw = hpool.tile([P, D], F32, tag="hraw", name="h_raw")
            nc.vector.tensor_add(h_raw[:], t2[:, 0, :], t2[:, 1, :])
            for h in (0, 1):
                pt = ps_s.tile([P, 512], F32, tag="ps", name="pt")
                nc.tensor.transpose(pt[:, :P], h_raw[:, h * P:(h + 1) * P],
                                    ident[:])
                nc.scalar.activation(hts[k][h][:], pt[:, :P],
                                     mybir.ActivationFunctionType.Relu,
                                     bias=b1t[h][:], scale=1.0)

        g8s = [gather_block(k) for k in range(KB)]
        finish_block(0, g8s[0])

        # ---- layer 2: stream over resident W2, 4 blocks -----------------
        vsub = [(v0, min(VT, VS - v0)) for v0 in range(0, VS, VT)]
        ei = 0
        for k in range(KB):
            for ti, (v0, vw) in enumerate(vsub):
                # emit the next block's tree/transpose/relu a little into
                # this block's matmul stream: its gathers are done by then,
                # and the PE transposes slot between matmul batches instead
                # of head-of-line blocking them.
                if ti == 2 and k + 1 < KB:
                    finish_block(k + 1, g8s[k + 1])
                po = ps_b.tile([P, VT], F32, tag="po", name="po")
                for sub in range(0, vw, 512):
                    sw = min(512, vw - sub)
                    for h in (0, 1):
                        nc.tensor.matmul(
                            po[:, sub:sub + sw],
                            lhsT=hts[k][h][:],
                            rhs=w2r[h][:, v0 + sub:v0 + sub + sw],
                            start=(h == 0),
                            stop=(h == 1))
                ot = opool.tile([P, VT], BF16, tag="ot", name="ot")
                if with_b2:
                    nc.vector.tensor_add(ot[:, :vw], po[:, :vw],
                                         b2bc[:, v0:v0 + vw])
                elif ei % 2 == 1:
                    nc.scalar.copy(ot[:, :vw], po[:, :vw])
                else:
                    nc.vector.tensor_copy(ot[:, :vw], po[:, :vw])
                ei += 1
                nc.sync.dma_start(
                    out_d[k * P:(k + 1) * P, v0:v0 + vw], ot[:, :vw])

    nc.compile()
    _CACHE[key] = nc
    return nc


def _host_prep(inputs, W1, b1, W2, b2):
    x = np.asarray(inputs)
    assert x.shape == (N, J) and x.dtype == np.int32

    # duplicate mask: scatter-SET semantics -> only first occurrence counts;
    # duplicates are redirected to the all-zero row V of the augmented W1T.
    dup = np.zeros((N, J), dtype=bool)
    for j in range(1, J):
        dup[:, j] = (x[:, :j] == x[:, j:j + 1]).any(axis=1)
    xd = np.where(dup, V, x).astype(np.int32)

    # idxq[q][p, k*J + j] = xd[(4q+k)*128 + p, j]   (batch quarter q)
    idxq = np.ascontiguousarray(
        xd.reshape(C // GV, KB, P, J).transpose(0, 2, 1, 3)
        .reshape(C // GV, P, KB * J))

    w1 = np.asarray(W1, dtype=np.float32)
    w1t = np.concatenate([w1.T * (1.0 / J), np.zeros((1, D), np.float32)],
                         axis=0)
    w1t = np.ascontiguousarray(w1t).astype(ml_dtypes.bfloat16)   # [V+1, D]

    w2t = np.ascontiguousarray(np.asarray(W2, dtype=np.float32).T)  # [D, V]
    w2t = w2t.astype(ml_dtypes.bfloat16)

    b1r = np.ascontiguousarray(np.asarray(b1, dtype=np.float32).reshape(2, P, 1))
    b2f = np.asarray(b2, dtype=np.float32)
    with_b2 = bool(np.any(b2f))

    in_maps = []
    for c in range(C):
        g, q = c // (C // GV), c % (C // GV)
        sl = slice(g * VS, (g + 1) * VS)
        m = {
            "idx": idxq[q],
            "w1t": w1t,
            "w2t": np.ascontiguousarray(w2t[:, sl]),
            "b1": b1r,
        }
        if with_b2:
            m["b2"] = np.ascontiguousarray(b2f[sl].reshape(1, VS))
        in_maps.append(m)
    return in_maps, with_b2


def run(inputs, W1, b1, W2, b2, trace=False):
    from concourse.bass_utils import run_bass_kernel_spmd

    in_maps, with_b2 = _host_prep(inputs, W1, b1, W2, b2)
    nc = _build(with_b2)
    res = run_bass_kernel_spmd(nc, in_maps, core_ids=list(range(C)), trace=trace)
    out = np.empty((N, V), dtype=ml_dtypes.bfloat16)
    for c in range(C):
        g, q = c // (C // GV), c % (C // GV)
        out[q * KB * P:(q + 1) * KB * P, g * VS:(g + 1) * VS] = \
            np.asarray(res.results[c]["out"])
    return out.astype(np.float32), res


def kernel(inputs, W1, b1, W2, b2):
    out, _ = run(inputs, W1, b1, W2, b2, trace=False)
    return out


# revision 20
# speedup vs baseline: 1.5793x; 1.0881x over previous
"""CBOW forward (embedding lookup -> ReLU -> vocab projection) on 8 TRN2 cores.

Full inputs in, full output out.  Sharding: 2 vocab halves x 4 batch
quarters.  Core c owns batch quarter Q = c % 4 (blocks 4Q..4Q+3, 512
examples) and vocab half g = c // 4 (columns [25000*g, 25000*(g+1))):
out[512*Q:512*(Q+1), 25000*g:25000*(g+1)] = relu(...) @ W2_g.T.

Why this split: the embedding gather is SWDGE descriptor-generation
bound (~1.4us per 128-row call, one gathered row per partition), so
per-core gather cost scales with the batch shard: a full-batch
(vocab-only-sharded) core needs 128 calls (~180us) and paces the
kernel; a quarter-batch core needs 32 (~45us), finished far ahead of
the matmul stream.  Meanwhile the vocab half keeps W2 resident in SBUF
(2 x [128, 25000] bf16 = ~98KB/partition), so the steady state moves
only the output (25.6MB bf16/core) - well under the ~366GB/s DMA
subsystem - and the TensorEngine stream (~2 x 200 matmul-512s at
~216ns warm cadence) becomes the binding resource.  The W2 load is
split into column segments so the first matmuls only wait on their own
segment.

Layer 1: 8 indirect row-gathers per 128-row block into an [n, 8, d]
bf16 SBUF tile (gather table bf16, pre-scaled by 1/(2*CTX) on host);
3-level DVE tree sum; PE transpose; ReLU(x + b1) on Scalar fused into
the PSUM eviction, emitting resident bf16 hT tiles.  Duplicate context
indices (scatter-SET semantics) are redirected on the host to an
appended all-zero row of W1T.

Layer 2 runs a single bf16 term per K-half (fp32 PSUM accumulate):
~5e-3 scale-relative error, inside the 2e-2 gate, at 1/3 the TensorE
cost of an fp32-grade split.  Output is stored bf16 (halving the
dominant write) and upcast to fp32 on host.  PSUM->SBUF evictions
round-robin over DVE+Scalar (+GpSimd once its gathers are done) so no
single engine paces the PE.  When b2 is nonzero (not the case here:
the problem zero-fills both biases) a variant with a resident bf16
broadcast b2 and DVE adds is compiled instead.
"""

from contextlib import ExitStack

import numpy as np
import ml_dtypes

import concourse.bacc as bacc
import concourse.bass as bass
import concourse.mybir as mybir
import concourse.tile as tile
from concourse.masks import make_identity

# Problem shape (hardcoded per the task contract).
N = 2048          # batch
J = 8             # context window (2*CTX)
D = 256           # hidden
V = 50000         # vocab
C = 8             # cores
P = 128
GV = 2             # vocab groups
KB = 4             # row-blocks per core (batch quarter = 512 examples)
VS = V // GV       # vocab columns per core = 25000
VT = 1024          # output tile width (two PSUM banks)
WSEG = 3125        # W2 resident-load column segment

F32 = mybir.dt.float32
BF16 = mybir.dt.bfloat16
I32 = mybir.dt.int32

_CACHE = {}


def _build(with_b2):
    """Build + compile the single-core SPMD Bass program."""
    key = ("nc", with_b2)
    if key in _CACHE:
        return _CACHE[key]

    nc = bacc.Bacc("TRN2", target_bir_lowering=False, debug=False, num_devices=C)

    idx_d = nc.dram_tensor("idx", [P, KB * J], I32, kind="ExternalInput")
    w1t_d = nc.dram_tensor("w1t", [V + 1, D], BF16, kind="ExternalInput")
    w2t_d = nc.dram_tensor("w2t", [D, VS], BF16, kind="ExternalInput")
    b1_d = nc.dram_tensor("b1", [2, P, 1], F32, kind="ExternalInput")
    if with_b2:
        b2_d = nc.dram_tensor("b2", [1, VS], F32, kind="ExternalInput")
    out_d = nc.dram_tensor("out", [KB * P, VS], BF16, kind="ExternalOutput")

    with tile.TileContext(nc) as tc, ExitStack() as ctx:
        const = ctx.enter_context(tc.tile_pool(name="const", bufs=1))
        gpool = ctx.enter_context(tc.tile_pool(name="g8", bufs=3))
        t4pool = ctx.enter_context(tc.tile_pool(name="t4", bufs=2))
        t2pool = ctx.enter_context(tc.tile_pool(name="t2", bufs=2))
        hpool = ctx.enter_context(tc.tile_pool(name="hraw", bufs=4))
        opool = ctx.enter_context(tc.tile_pool(name="out", bufs=6))
        ps_s = ctx.enter_context(tc.tile_pool(name="ps_s", bufs=2, space="PSUM"))
        ps_b = ctx.enter_context(tc.tile_pool(name="ps_b", bufs=3, space="PSUM"))

        # ---- resident tensors -------------------------------------------
        idx_sb = const.tile([P, KB * J], I32, tag="idx")
        nc.sync.dma_start(idx_sb[:], idx_d[:])
        ident = const.tile([P, P], F32, tag="ident")
        make_identity(nc, ident[:])
        b1t = [const.tile([P, 1], F32, tag=f"b1{h}", name=f"b1{h}")
               for h in (0, 1)]
        for h in (0, 1):
            nc.sync.dma_start(b1t[h][:], b1_d[h])
        # resident W2 half, loaded in column segments (early matmuls only
        # wait on their own segment thanks to subtile dependency tracking)
        w2r = [const.tile([P, VS], BF16, tag=f"w2r{h}", name=f"w2r{h}")
               for h in (0, 1)]
        for s0 in range(0, VS, WSEG):
            sw = min(WSEG, VS - s0)
            for h in (0, 1):
                nc.sync.dma_start(w2r[h][:, s0:s0 + sw],
                                  w2t_d[h * P:(h + 1) * P, s0:s0 + sw])
        if with_b2:
            b2bc = const.tile([P, VS], BF16, tag="b2bc")
            nc.sync.dma_start(b2bc[:], b2_d[:].to_broadcast([P, VS]))
        # resident hT tiles for the core's KB blocks
        hts = [[const.tile([P, P], BF16, tag=f"ht{k}{h}", name=f"ht{k}{h}")
                for h in (0, 1)] for k in range(KB)]

        # ---- layer 1 ----------------------------------------------------
        def gather_block(k):
            # gathers + tree sum all on gpsimd: the tree runs in FIFO order
            # right after its own block's gathers with zero cross-queue
            # latency (gpsimd has nothing else to do), at the cost of
            # delaying the next block's gathers by ~3us - irrelevant, the
            # gather pipeline has huge slack vs the PE stream.
            g8 = gpool.tile([P, J, D], BF16, tag="g8", name="g8")
            for j in range(J):
                nc.gpsimd.indirect_dma_start(
                    out=g8[:, j, :],
                    out_offset=None,
                    in_=w1t_d[:],
                    in_offset=bass.IndirectOffsetOnAxis(
                        ap=idx_sb[:, k * J + j:k * J + j + 1], axis=0),
                )
            t4 = t4pool.tile([P, 4, D], BF16, tag="t4", name="t4")
            nc.gpsimd.tensor_add(t4[:], g8[:, 0:4, :], g8[:, 4:8, :])
            t2 = t2pool.tile([P, 2, D], BF16, tag="t2", name="t2")
            nc.gpsimd.tensor_add(t2[:], t4[:, 0:2, :], t4[:, 2:4, :])
            h_raw = hpool.tile([P, D], F32, tag="hraw", name="h_raw")
            nc.gpsimd.tensor_add(h_raw[:], t2[:, 0, :], t2[:, 1, :])
            return h_raw

        def finish_block(k, h_raw):
            for h in (0, 1):
                pt = ps_s.tile([P, 512], F32, tag="ps", name="pt")
                nc.tensor.transpose(pt[:, :P], h_raw[:, h * P:(h + 1) * P],
                                    ident[:])
                nc.scalar.activation(hts[k][h][:], pt[:, :P],
                                     mybir.ActivationFunctionType.Relu,
                                     bias=b1t[h][:], scale=1.0)

        hraws = [gather_block(k) for k in range(KB)]
        finish_block(0, hraws[0])

        # ---- layer 2: stream over resident W2, 4 blocks -----------------
        vsub = [(v0, min(VT, VS - v0)) for v0 in range(0, VS, VT)]
        ei = 0
        for k in range(KB):
            for ti, (v0, vw) in enumerate(vsub):
                # emit the next block's transpose/relu late in this block's
                # matmul stream: by then its gather+tree (gpsimd) are done,
                # so the PE transposes slot between matmul batches without
                # head-of-line blocking ready matmuls behind them.
                if ti == 18 and k + 1 < KB:
                    finish_block(k + 1, hraws[k + 1])
                po = ps_b.tile([P, VT], F32, tag="po", name="po")
                for sub in range(0, vw, 512):
                    sw = min(512, vw - sub)
                    for h in (0, 1):
                        nc.tensor.matmul(
                            po[:, sub:sub + sw],
                            lhsT=hts[k][h][:],
                            rhs=w2r[h][:, v0 + sub:v0 + sub + sw],
                            start=(h == 0),
                            stop=(h == 1))
                ot = opool.tile([P, VT], BF16, tag="ot", name="ot")
                if with_b2:
                    nc.vector.tensor_add(ot[:, :vw], po[:, :vw],
                                         b2bc[:, v0:v0 + vw])
                elif ei % 2 == 1:
                    nc.scalar.copy(ot[:, :vw], po[:, :vw])
                else:
                    nc.vector.tensor_copy(ot[:, :vw], po[:, :vw])
                ei += 1
                nc.sync.dma_start(
                    out_d[k * P:(k + 1) * P, v0:v0 + vw], ot[:, :vw])

    nc.compile()
    _CACHE[key] = nc
    return nc


def _host_prep(inputs, W1, b1, W2, b2):
    x = np.asarray(inputs)
    assert x.shape == (N, J) and x.dtype == np.int32

    # duplicate mask: scatter-SET semantics -> only first occurrence counts;
    # duplicates are redirected to the all-zero row V of the augmented W1T.
    dup = np.zeros((N, J), dtype=bool)
    for j in range(1, J):
        dup[:, j] = (x[:, :j] == x[:, j:j + 1]).any(axis=1)
    xd = np.where(dup, V, x).astype(np.int32)

    # idxq[q][p, k*J + j] = xd[(4q+k)*128 + p, j]   (batch quarter q)
    idxq = np.ascontiguousarray(
        xd.reshape(C // GV, KB, P, J).transpose(0, 2, 1, 3)
        .reshape(C // GV, P, KB * J))

    w1 = np.asarray(W1, dtype=np.float32)
    w1t = np.concatenate([w1.T * (1.0 / J), np.zeros((1, D), np.float32)],
                         axis=0)
    w1t = np.ascontiguousarray(w1t).astype(ml_dtypes.bfloat16)   # [V+1, D]

    w2t = np.ascontiguousarray(np.asarray(W2, dtype=np.float32).T)  # [D, V]
    w2t = w2t.astype(ml_dtypes.bfloat16)

    b1r = np.ascontiguousarray(np.asarray(b1, dtype=np.float32).reshape(2, P, 1))
    b2f = np.asarray(b2, dtype=np.float32)
    with_b2 = bool(np.any(b2f))

    in_maps = []
    for c in range(C):
        g, q = c // (C // GV), c % (C // GV)
        sl = slice(g * VS, (g + 1) * VS)
        m = {
            "idx": idxq[q],
            "w1t": w1t,
            "w2t": np.ascontiguousarray(w2t[:, sl]),
            "b1": b1r,
        }
        if with_b2:
            m["b2"] = np.ascontiguousarray(b2f[sl].reshape(1, VS))
        in_maps.append(m)
    return in_maps, with_b2


def run(inputs, W1, b1, W2, b2, trace=False):
    from concourse.bass_utils import run_bass_kernel_spmd

    in_maps, with_b2 = _host_prep(inputs, W1, b1, W2, b2)
    nc = _build(with_b2)
    res = run_bass_kernel_spmd(nc, in_maps, core_ids=list(range(C)), trace=trace)
    out = np.empty((N, V), dtype=ml_dtypes.bfloat16)
    for c in range(C):
        g, q = c // (C // GV), c % (C // GV)
        out[q * KB * P:(q + 1) * KB * P, g * VS:(g + 1) * VS] = \
            np.asarray(res.results[c]["out"])
    return out.astype(np.float32), res


def kernel(inputs, W1, b1, W2, b2):
    out, _ = run(inputs, W1, b1, W2, b2, trace=False)
    return out


# revision 22
# speedup vs baseline: 1.6304x; 1.0323x over previous
"""CBOW forward (embedding lookup -> ReLU -> vocab projection) on 8 TRN2 cores.

Full inputs in, full output out.  Sharding: 2 vocab halves x 4 batch
quarters.  Core c owns batch quarter Q = c % 4 (blocks 4Q..4Q+3, 512
examples) and vocab half g = c // 4 (columns [25000*g, 25000*(g+1))):
out[512*Q:512*(Q+1), 25000*g:25000*(g+1)] = relu(...) @ W2_g.T.

Why this split: the embedding gather is SWDGE descriptor-generation
bound (~1.4us per 128-row call, one gathered row per partition), so
per-core gather cost scales with the batch shard: a full-batch
(vocab-only-sharded) core needs 128 calls (~180us) and paces the
kernel; a quarter-batch core needs 32 (~45us), finished far ahead of
the matmul stream.  Meanwhile the vocab half keeps W2 resident in SBUF
(2 x [128, 25000] bf16 = ~98KB/partition), so the steady state moves
only the output (25.6MB bf16/core) - well under the ~366GB/s DMA
subsystem - and the TensorEngine stream (~2 x 200 matmul-512s at
~216ns warm cadence) becomes the binding resource.  The W2 load is
split into column segments so the first matmuls only wait on their own
segment.

Layer 1: 8 indirect row-gathers per 128-row block into an [n, 8, d]
bf16 SBUF tile (gather table bf16, pre-scaled by 1/(2*CTX) on host);
3-level DVE tree sum; PE transpose; ReLU(x + b1) on Scalar fused into
the PSUM eviction, emitting resident bf16 hT tiles.  Duplicate context
indices (scatter-SET semantics) are redirected on the host to an
appended all-zero row of W1T.

Layer 2 runs a single bf16 term per K-half (fp32 PSUM accumulate):
~5e-3 scale-relative error, inside the 2e-2 gate, at 1/3 the TensorE
cost of an fp32-grade split.  Output is stored bf16 (halving the
dominant write) and upcast to fp32 on host.  PSUM->SBUF evictions
round-robin over DVE+Scalar (+GpSimd once its gathers are done) so no
single engine paces the PE.  When b2 is nonzero (not the case here:
the problem zero-fills both biases) a variant with a resident bf16
broadcast b2 and DVE adds is compiled instead.
"""

from contextlib import ExitStack

import numpy as np
import ml_dtypes

import concourse.bacc as bacc
import concourse.bass as bass
import concourse.mybir as mybir
import concourse.tile as tile
from concourse.masks import make_identity

# Problem shape (hardcoded per the task contract).
N = 2048          # batch
J = 8             # context window (2*CTX)
D = 256           # hidden
V = 50000         # vocab
C = 8             # cores
P = 128
GV = 2             # vocab groups
KB = 4             # row-blocks per core (batch quarter = 512 examples)
VS = V // GV       # vocab columns per core = 25000
VT = 1024          # output tile width (two PSUM banks)
WSEG = 3125        # W2 resident-load column segment

F32 = mybir.dt.float32
BF16 = mybir.dt.bfloat16
I32 = mybir.dt.int32

_CACHE = {}


def _build(with_b2):
    """Build + compile the single-core SPMD Bass program."""
    key = ("nc", with_b2)
    if key in _CACHE:
        return _CACHE[key]

    nc = bacc.Bacc("TRN2", target_bir_lowering=False, debug=False, num_devices=C)

    idx_d = nc.dram_tensor("idx", [P, KB * J], I32, kind="ExternalInput")
    w1t_d = nc.dram_tensor("w1t", [V + 1, D], BF16, kind="ExternalInput")
    w2t_d = nc.dram_tensor("w2t", [D, VS], BF16, kind="ExternalInput")
    b1_d = nc.dram_tensor("b1", [2, P, 1], F32, kind="ExternalInput")
    if with_b2:
        b2_d = nc.dram_tensor("b2", [1, VS], F32, kind="ExternalInput")
    out_d = nc.dram_tensor("out", [KB * P, VS], BF16, kind="ExternalOutput")

    with tile.TileContext(nc) as tc, ExitStack() as ctx:
        const = ctx.enter_context(tc.tile_pool(name="const", bufs=1))
        gpool = ctx.enter_context(tc.tile_pool(name="g8", bufs=3))
        t4pool = ctx.enter_context(tc.tile_pool(name="t4", bufs=2))
        t2pool = ctx.enter_context(tc.tile_pool(name="t2", bufs=2))
        hpool = ctx.enter_context(tc.tile_pool(name="hraw", bufs=4))
        opool = ctx.enter_context(tc.tile_pool(name="out", bufs=6))
        ps_s = ctx.enter_context(tc.tile_pool(name="ps_s", bufs=2, space="PSUM"))
        ps_b = ctx.enter_context(tc.tile_pool(name="ps_b", bufs=3, space="PSUM"))

        # ---- resident tensors -------------------------------------------
        idx_sb = const.tile([P, KB * J], I32, tag="idx")
        nc.sync.dma_start(idx_sb[:], idx_d[:])
        ident = const.tile([P, P], F32, tag="ident")
        make_identity(nc, ident[:])
        b1t = [const.tile([P, 1], F32, tag=f"b1{h}", name=f"b1{h}")
               for h in (0, 1)]
        for h in (0, 1):
            nc.sync.dma_start(b1t[h][:], b1_d[h])
        # resident W2 half, loaded in column segments (early matmuls only
        # wait on their own segment thanks to subtile dependency tracking)
        w2r = [const.tile([P, VS], BF16, tag=f"w2r{h}", name=f"w2r{h}")
               for h in (0, 1)]
        for s0 in range(0, VS, WSEG):
            sw = min(WSEG, VS - s0)
            for h in (0, 1):
                nc.sync.dma_start(w2r[h][:, s0:s0 + sw],
                                  w2t_d[h * P:(h + 1) * P, s0:s0 + sw])
        if with_b2:
            b2bc = const.tile([P, VS], BF16, tag="b2bc")
            nc.sync.dma_start(b2bc[:], b2_d[:].to_broadcast([P, VS]))
        # resident hT tiles for the core's KB blocks
        hts = [[const.tile([P, P], BF16, tag=f"ht{k}{h}", name=f"ht{k}{h}")
                for h in (0, 1)] for k in range(KB)]

        # ---- layer 1 ----------------------------------------------------
        def gather_block(k):
            # gathers only on gpsimd (its elementwise ops are ~2.5x slower
            # than DVE, so the tree does NOT live here)
            g8 = gpool.tile([P, J, D], BF16, tag="g8", name="g8")
            for j in range(J):
                nc.gpsimd.indirect_dma_start(
                    out=g8[:, j, :],
                    out_offset=None,
                    in_=w1t_d[:],
                    in_offset=bass.IndirectOffsetOnAxis(
                        ap=idx_sb[:, k * J + j:k * J + j + 1], axis=0),
                )
            return g8

        def finish_block(k, g8):
            # 3-level DVE tree sum, PE transpose, Scalar relu(x+b1) -> bf16.
            # Emitted mid-stream of the previous block, where the DVE queue
            # holds at most a couple of evictions, so the whole chain
            # completes within a few tiles.
            t4 = t4pool.tile([P, 4, D], BF16, tag="t4", name="t4")
            nc.vector.tensor_add(t4[:], g8[:, 0:4, :], g8[:, 4:8, :])
            t2 = t2pool.tile([P, 2, D], BF16, tag="t2", name="t2")
            nc.vector.tensor_add(t2[:], t4[:, 0:2, :], t4[:, 2:4, :])
            h_raw = hpool.tile([P, D], F32, tag="hraw", name="h_raw")
            nc.vector.tensor_add(h_raw[:], t2[:, 0, :], t2[:, 1, :])
            for h in (0, 1):
                pt = ps_s.tile([P, 512], F32, tag="ps", name="pt")
                nc.tensor.transpose(pt[:, :P], h_raw[:, h * P:(h + 1) * P],
                                    ident[:])
                nc.scalar.activation(hts[k][h][:], pt[:, :P],
                                     mybir.ActivationFunctionType.Relu,
                                     bias=b1t[h][:], scale=1.0)

        g8s = [gather_block(k) for k in range(KB)]
        finish_block(0, g8s[0])

        # ---- layer 2: stream over resident W2, 4 blocks -----------------
        vsub = [(v0, min(VT, VS - v0)) for v0 in range(0, VS, VT)]
        ei = 0
        for k in range(KB):
            for ti, (v0, vw) in enumerate(vsub):
                # emit the next block's tree/transpose/relu late in this
                # block's matmul stream: by then its gathers are long done,
                # so the chain completes in a few tiles and the PE
                # transposes slot between matmul batches without
                # head-of-line blocking ready matmuls behind them.
                if ti == 16 and k + 1 < KB:
                    finish_block(k + 1, g8s[k + 1])
                po = ps_b.tile([P, VT], F32, tag="po", name="po")
                for sub in range(0, vw, 512):
                    sw = min(512, vw - sub)
                    for h in (0, 1):
                        nc.tensor.matmul(
                            po[:, sub:sub + sw],
                            lhsT=hts[k][h][:],
                            rhs=w2r[h][:, v0 + sub:v0 + sub + sw],
                            start=(h == 0),
                            stop=(h == 1))
                ot = opool.tile([P, VT], BF16, tag="ot", name="ot")
                if with_b2:
                    nc.vector.tensor_add(ot[:, :vw], po[:, :vw],
                                         b2bc[:, v0:v0 + vw])
                elif ei % 2 == 1:
                    nc.scalar.copy(ot[:, :vw], po[:, :vw])
                else:
                    nc.vector.tensor_copy(ot[:, :vw], po[:, :vw])
                ei += 1
                nc.sync.dma_start(
                    out_d[k * P:(k + 1) * P, v0:v0 + vw], ot[:, :vw])

    nc.compile()
    _CACHE[key] = nc
    return nc


def _host_prep(inputs, W1, b1, W2, b2):
    x = np.asarray(inputs)
    assert x.shape == (N, J) and x.dtype == np.int32

    # duplicate mask: scatter-SET semantics -> only first occurrence counts;
    # duplicates are redirected to the all-zero row V of the augmented W1T.
    dup = np.zeros((N, J), dtype=bool)
    for j in range(1, J):
        dup[:, j] = (x[:, :j] == x[:, j:j + 1]).any(axis=1)
    xd = np.where(dup, V, x).astype(np.int32)

    # idxq[q][p, k*J + j] = xd[(4q+k)*128 + p, j]   (batch quarter q)
    idxq = np.ascontiguousarray(
        xd.reshape(C // GV, KB, P, J).transpose(0, 2, 1, 3)
        .reshape(C // GV, P, KB * J))

    w1 = np.asarray(W1, dtype=np.float32)
    w1t = np.concatenate([w1.T * (1.0 / J), np.zeros((1, D), np.float32)],
                         axis=0)
    w1t = np.ascontiguousarray(w1t).astype(ml_dtypes.bfloat16)   # [V+1, D]

    w2t = np.ascontiguousarray(np.asarray(W2, dtype=np.float32).T)  # [D, V]
    w2t = w2t.astype(ml_dtypes.bfloat16)

    b1r = np.ascontiguousarray(np.asarray(b1, dtype=np.float32).reshape(2, P, 1))
    b2f = np.asarray(b2, dtype=np.float32)
    with_b2 = bool(np.any(b2f))

    in_maps = []
    for c in range(C):
        g, q = c // (C // GV), c % (C // GV)
        sl = slice(g * VS, (g + 1) * VS)
        m = {
            "idx": idxq[q],
            "w1t": w1t,
            "w2t": np.ascontiguousarray(w2t[:, sl]),
            "b1": b1r,
        }
        if with_b2:
            m["b2"] = np.ascontiguousarray(b2f[sl].reshape(1, VS))
        in_maps.append(m)
    return in_maps, with_b2


def run(inputs, W1, b1, W2, b2, trace=False):
    from concourse.bass_utils import run_bass_kernel_spmd

    in_maps, with_b2 = _host_prep(inputs, W1, b1, W2, b2)
    nc = _build(with_b2)
    res = run_bass_kernel_spmd(nc, in_maps, core_ids=list(range(C)), trace=trace)
    out = np.empty((N, V), dtype=ml_dtypes.bfloat16)
    for c in range(C):
        g, q = c // (C // GV), c % (C // GV)
        out[q * KB * P:(q + 1) * KB * P, g * VS:(g + 1) * VS] = \
            np.asarray(res.results[c]["out"])
    return out.astype(np.float32), res


def kernel(inputs, W1, b1, W2, b2):
    out, _ = run(inputs, W1, b1, W2, b2, trace=False)
    return out


# revision 25
# speedup vs baseline: 1.6709x; 1.0249x over previous
"""CBOW forward (embedding lookup -> ReLU -> vocab projection) on 8 TRN2 cores.

Full inputs in, full output out.  Sharding: 2 vocab halves x 4 batch
quarters.  Core c owns batch quarter Q = c % 4 (blocks 4Q..4Q+3, 512
examples) and vocab half g = c // 4 (columns [25000*g, 25000*(g+1))):
out[512*Q:512*(Q+1), 25000*g:25000*(g+1)] = relu(...) @ W2_g.T.

Why this split: the embedding gather is SWDGE descriptor-generation
bound (~1.4us per 128-row call, one gathered row per partition), so
per-core gather cost scales with the batch shard: a full-batch
(vocab-only-sharded) core needs 128 calls (~180us) and paces the
kernel; a quarter-batch core needs 32 (~45us), finished far ahead of
the matmul stream.  Meanwhile the vocab half keeps W2 resident in SBUF
(2 x [128, 25000] bf16 = ~98KB/partition), so the steady state moves
only the output (25.6MB bf16/core) - well under the ~366GB/s DMA
subsystem - and the TensorEngine stream (~2 x 200 matmul-512s at
~216ns warm cadence) becomes the binding resource.  The W2 load is
split into column segments so the first matmuls only wait on their own
segment.

Layer 1: 8 indirect row-gathers per 128-row block into an [n, 8, d]
bf16 SBUF tile (gather table bf16, pre-scaled by 1/(2*CTX) on host);
3-level DVE tree sum; PE transpose; ReLU(x + b1) on Scalar fused into
the PSUM eviction, emitting resident bf16 hT tiles.  Duplicate context
indices (scatter-SET semantics) are redirected on the host to an
appended all-zero row of W1T.

Layer 2 runs a single bf16 term per K-half (fp32 PSUM accumulate):
~5e-3 scale-relative error, inside the 2e-2 gate, at 1/3 the TensorE
cost of an fp32-grade split.  Output is stored bf16 (halving the
dominant write) and upcast to fp32 on host.  PSUM->SBUF evictions
round-robin over DVE+Scalar (+GpSimd once its gathers are done) so no
single engine paces the PE.  When b2 is nonzero (not the case here:
the problem zero-fills both biases) a variant with a resident bf16
broadcast b2 and DVE adds is compiled instead.
"""

from contextlib import ExitStack

import numpy as np
import ml_dtypes

import concourse.bacc as bacc
import concourse.bass as bass
import concourse.mybir as mybir
import concourse.tile as tile
from concourse.masks import make_identity

# Problem shape (hardcoded per the task contract).
N = 2048          # batch
J = 8             # context window (2*CTX)
D = 256           # hidden
V = 50000         # vocab
C = 8             # cores
P = 128
GV = 2             # vocab groups
KB = 4             # row-blocks per core (batch quarter = 512 examples)
VS = V // GV       # vocab columns per core = 25000
VT = 1024          # output tile width (two PSUM banks)
WSEG = 3125        # W2 resident-load column segment

F32 = mybir.dt.float32
BF16 = mybir.dt.bfloat16
I32 = mybir.dt.int32

_CACHE = {}


def _build(with_b2):
    """Build + compile the single-core SPMD Bass program."""
    key = ("nc", with_b2)
    if key in _CACHE:
        return _CACHE[key]

    nc = bacc.Bacc("TRN2", target_bir_lowering=False, debug=False, num_devices=C)

    idx_d = nc.dram_tensor("idx", [P, KB * J], I32, kind="ExternalInput")
    w1t_d = nc.dram_tensor("w1t", [V + 1, D], BF16, kind="ExternalInput")
    w2t_d = nc.dram_tensor("w2t", [D, VS], BF16, kind="ExternalInput")
    b1_d = nc.dram_tensor("b1", [2, P, 1], F32, kind="ExternalInput")
    if with_b2:
        b2_d = nc.dram_tensor("b2", [1, VS], F32, kind="ExternalInput")
    out_d = nc.dram_tensor("out", [KB * P, VS], BF16, kind="ExternalOutput")

    with tile.TileContext(nc) as tc, ExitStack() as ctx:
        const = ctx.enter_context(tc.tile_pool(name="const", bufs=1))
        gpool = ctx.enter_context(tc.tile_pool(name="g8", bufs=3))
        t4pool = ctx.enter_context(tc.tile_pool(name="t4", bufs=2))
        t2pool = ctx.enter_context(tc.tile_pool(name="t2", bufs=2))
        hpool = ctx.enter_context(tc.tile_pool(name="hraw", bufs=4))
        opool = ctx.enter_context(tc.tile_pool(name="out", bufs=31))
        ps_s = ctx.enter_context(tc.tile_pool(name="ps_s", bufs=2, space="PSUM"))
        ps_b = ctx.enter_context(tc.tile_pool(name="ps_b", bufs=3, space="PSUM"))

        # ---- resident tensors -------------------------------------------
        idx_sb = const.tile([P, KB * J], I32, tag="idx")
        nc.sync.dma_start(idx_sb[:], idx_d[:])
        ident = const.tile([P, P], F32, tag="ident")
        make_identity(nc, ident[:])
        b1t = [const.tile([P, 1], F32, tag=f"b1{h}", name=f"b1{h}")
               for h in (0, 1)]
        for h in (0, 1):
            nc.sync.dma_start(b1t[h][:], b1_d[h])
        # resident W2 half, loaded in column segments (early matmuls only
        # wait on their own segment thanks to subtile dependency tracking)
        w2r = [const.tile([P, VS], BF16, tag=f"w2r{h}", name=f"w2r{h}")
               for h in (0, 1)]
        for s0 in range(0, VS, WSEG):
            sw = min(WSEG, VS - s0)
            for h in (0, 1):
                nc.sync.dma_start(w2r[h][:, s0:s0 + sw],
                                  w2t_d[h * P:(h + 1) * P, s0:s0 + sw])
        if with_b2:
            b2bc = const.tile([P, VS], BF16, tag="b2bc")
            nc.sync.dma_start(b2bc[:], b2_d[:].to_broadcast([P, VS]))
        # resident hT tiles for the core's KB blocks
        hts = [[const.tile([P, P], BF16, tag=f"ht{k}{h}", name=f"ht{k}{h}")
                for h in (0, 1)] for k in range(KB)]

        # ---- layer 1 ----------------------------------------------------
        def gather_block(k):
            # gathers only on gpsimd (its elementwise ops are ~2.5x slower
            # than DVE, so the tree does NOT live here)
            g8 = gpool.tile([P, J, D], BF16, tag="g8", name="g8")
            for j in range(J):
                nc.gpsimd.indirect_dma_start(
                    out=g8[:, j, :],
                    out_offset=None,
                    in_=w1t_d[:],
                    in_offset=bass.IndirectOffsetOnAxis(
                        ap=idx_sb[:, k * J + j:k * J + j + 1], axis=0),
                )
            return g8

        def finish_block(k, g8):
            # 3-level DVE tree sum, PE transpose, Scalar relu(x+b1) -> bf16.
            # Emitted mid-stream of the previous block, where the DVE queue
            # holds at most a couple of evictions, so the whole chain
            # completes within a few tiles.
            t4 = t4pool.tile([P, 4, D], BF16, tag="t4", name="t4")
            nc.vector.tensor_add(t4[:], g8[:, 0:4, :], g8[:, 4:8, :])
            t2 = t2pool.tile([P, 2, D], BF16, tag="t2", name="t2")
            nc.vector.tensor_add(t2[:], t4[:, 0:2, :], t4[:, 2:4, :])
            h_raw = hpool.tile([P, D], F32, tag="hraw", name="h_raw")
            nc.vector.tensor_add(h_raw[:], t2[:, 0, :], t2[:, 1, :])
            for h in (0, 1):
                pt = ps_s.tile([P, 512], F32, tag="ps", name="pt")
                nc.tensor.transpose(pt[:, :P], h_raw[:, h * P:(h + 1) * P],
                                    ident[:])
                nc.scalar.activation(hts[k][h][:], pt[:, :P],
                                     mybir.ActivationFunctionType.Relu,
                                     bias=b1t[h][:], scale=1.0)

        g8s = [gather_block(k) for k in range(KB)]
        finish_block(0, g8s[0])

        # ---- layer 2: stream over resident W2, 4 blocks -----------------
        vsub = [(v0, min(VT, VS - v0)) for v0 in range(0, VS, VT)]
        ei = 0
        deferred = []  # block 0's output DMAs, issued during block 1
        for k in range(KB):
            for ti, (v0, vw) in enumerate(vsub):
                # flush one deferred block-0 output write per tile: during
                # block 0 the sync queue issues no output DMAs at all, so
                # the resident-W2 segment loads get the full DMA bandwidth
                # instead of contending with the output stream.
                if k >= 1 and deferred:
                    dst, src = deferred.pop(0)
                    nc.sync.dma_start(dst, src)
                # emit the next block's tree/transpose/relu late in this
                # block's matmul stream: by then its gathers are long done,
                # so the chain completes in a few tiles and the PE
                # transposes slot between matmul batches without
                # head-of-line blocking ready matmuls behind them.
                if ti == 16 and k + 1 < KB:
                    finish_block(k + 1, g8s[k + 1])
                po = ps_b.tile([P, VT], F32, tag="po", name="po")
                for sub in range(0, vw, 512):
                    sw = min(512, vw - sub)
                    for h in (0, 1):
                        nc.tensor.matmul(
                            po[:, sub:sub + sw],
                            lhsT=hts[k][h][:],
                            rhs=w2r[h][:, v0 + sub:v0 + sub + sw],
                            start=(h == 0),
                            stop=(h == 1))
                ot = opool.tile([P, VT], BF16, tag="ot", name="ot")
                if with_b2:
                    nc.vector.tensor_add(ot[:, :vw], po[:, :vw],
                                         b2bc[:, v0:v0 + vw])
                elif ei % 2 == 1:
                    nc.scalar.copy(ot[:, :vw], po[:, :vw])
                else:
                    nc.vector.tensor_copy(ot[:, :vw], po[:, :vw])
                ei += 1
                if k == 0:
                    deferred.append(
                        (out_d[0:P, v0:v0 + vw], ot[:, :vw]))
                else:
                    nc.sync.dma_start(
                        out_d[k * P:(k + 1) * P, v0:v0 + vw], ot[:, :vw])
        assert not deferred

    nc.compile()
    _CACHE[key] = nc
    return nc


def _host_prep(inputs, W1, b1, W2, b2):
    x = np.asarray(inputs)
    assert x.shape == (N, J) and x.dtype == np.int32

    # duplicate mask: scatter-SET semantics -> only first occurrence counts;
    # duplicates are redirected to the all-zero row V of the augmented W1T.
    dup = np.zeros((N, J), dtype=bool)
    for j in range(1, J):
        dup[:, j] = (x[:, :j] == x[:, j:j + 1]).any(axis=1)
    xd = np.where(dup, V, x).astype(np.int32)

    # idxq[q][p, k*J + j] = xd[(4q+k)*128 + p, j]   (batch quarter q)
    idxq = np.ascontiguousarray(
        xd.reshape(C // GV, KB, P, J).transpose(0, 2, 1, 3)
        .reshape(C // GV, P, KB * J))

    w1 = np.asarray(W1, dtype=np.float32)
    w1t = np.concatenate([w1.T * (1.0 / J), np.zeros((1, D), np.float32)],
                         axis=0)
    w1t = np.ascontiguousarray(w1t).astype(ml_dtypes.bfloat16)   # [V+1, D]

    w2t = np.ascontiguousarray(np.asarray(W2, dtype=np.float32).T)  # [D, V]
    w2t = w2t.astype(ml_dtypes.bfloat16)

    b1r = np.ascontiguousarray(np.asarray(b1, dtype=np.float32).reshape(2, P, 1))
    b2f = np.asarray(b2, dtype=np.float32)
    with_b2 = bool(np.any(b2f))

    in_maps = []
    for c in range(C):
        g, q = c // (C // GV), c % (C // GV)
        sl = slice(g * VS, (g + 1) * VS)
        m = {
            "idx": idxq[q],
            "w1t": w1t,
            "w2t": np.ascontiguousarray(w2t[:, sl]),
            "b1": b1r,
        }
        if with_b2:
            m["b2"] = np.ascontiguousarray(b2f[sl].reshape(1, VS))
        in_maps.append(m)
    return in_maps, with_b2


def run(inputs, W1, b1, W2, b2, trace=False):
    from concourse.bass_utils import run_bass_kernel_spmd

    in_maps, with_b2 = _host_prep(inputs, W1, b1, W2, b2)
    nc = _build(with_b2)
    res = run_bass_kernel_spmd(nc, in_maps, core_ids=list(range(C)), trace=trace)
    out = np.empty((N, V), dtype=ml_dtypes.bfloat16)
    for c in range(C):
        g, q = c // (C // GV), c % (C // GV)
        out[q * KB * P:(q + 1) * KB * P, g * VS:(g + 1) * VS] = \
            np.asarray(res.results[c]["out"])
    return out.astype(np.float32), res


def kernel(inputs, W1, b1, W2, b2):
    out, _ = run(inputs, W1, b1, W2, b2, trace=False)
    return out


# revision 27
# speedup vs baseline: 1.6722x; 1.0007x over previous
"""CBOW forward (embedding lookup -> ReLU -> vocab projection) on 8 TRN2 cores.

Full inputs in, full output out.  Sharding: 2 vocab halves x 4 batch
quarters.  Core c owns batch quarter Q = c % 4 (blocks 4Q..4Q+3, 512
examples) and vocab half g = c // 4 (columns [25000*g, 25000*(g+1))):
out[512*Q:512*(Q+1), 25000*g:25000*(g+1)] = relu(...) @ W2_g.T.

Why this split: the embedding gather is SWDGE descriptor-generation
bound (~1.4us per 128-row call, one gathered row per partition), so
per-core gather cost scales with the batch shard: a full-batch
(vocab-only-sharded) core needs 128 calls (~180us) and paces the
kernel; a quarter-batch core needs 32 (~45us), finished far ahead of
the matmul stream.  Meanwhile the vocab half keeps W2 resident in SBUF
(2 x [128, 25000] bf16 = ~98KB/partition), so the steady state moves
only the output (25.6MB bf16/core) - well under the ~366GB/s DMA
subsystem - and the TensorEngine stream (~2 x 200 matmul-512s at
~216ns warm cadence) becomes the binding resource.  The W2 load is
split into column segments so the first matmuls only wait on their own
segment.

Layer 1: 8 indirect row-gathers per 128-row block into an [n, 8, d]
bf16 SBUF tile (gather table bf16, pre-scaled by 1/(2*CTX) on host);
3-level DVE tree sum; PE transpose; ReLU(x + b1) on Scalar fused into
the PSUM eviction, emitting resident bf16 hT tiles.  Duplicate context
indices (scatter-SET semantics) are redirected on the host to an
appended all-zero row of W1T.

Layer 2 runs a single bf16 term per K-half (fp32 PSUM accumulate):
~5e-3 scale-relative error, inside the 2e-2 gate, at 1/3 the TensorE
cost of an fp32-grade split.  Output is stored bf16 (halving the
dominant write) and upcast to fp32 on host.  PSUM->SBUF evictions
round-robin over DVE+Scalar (+GpSimd once its gathers are done) so no
single engine paces the PE.  When b2 is nonzero (not the case here:
the problem zero-fills both biases) a variant with a resident bf16
broadcast b2 and DVE adds is compiled instead.
"""

from contextlib import ExitStack

import numpy as np
import ml_dtypes

import concourse.bacc as bacc
import concourse.bass as bass
import concourse.mybir as mybir
import concourse.tile as tile
from concourse.masks import make_identity

# Problem shape (hardcoded per the task contract).
N = 2048          # batch
J = 8             # context window (2*CTX)
D = 256           # hidden
V = 50000         # vocab
C = 8             # cores
P = 128
GV = 2             # vocab groups
KB = 4             # row-blocks per core (batch quarter = 512 examples)
VS = V // GV       # vocab columns per core = 25000
VT = 1024          # output tile width (two PSUM banks)
WSEG = 3125        # W2 resident-load column segment

F32 = mybir.dt.float32
BF16 = mybir.dt.bfloat16
I32 = mybir.dt.int32

_CACHE = {}


def _build(with_b2):
    """Build + compile the single-core SPMD Bass program."""
    key = ("nc", with_b2)
    if key in _CACHE:
        return _CACHE[key]

    nc = bacc.Bacc("TRN2", target_bir_lowering=False, debug=False, num_devices=C)

    idx_d = nc.dram_tensor("idx", [P, KB * J], I32, kind="ExternalInput")
    w1t_d = nc.dram_tensor("w1t", [V + 1, D], BF16, kind="ExternalInput")
    w2t_d = nc.dram_tensor("w2t", [D, VS], BF16, kind="ExternalInput")
    b1_d = nc.dram_tensor("b1", [2, P, 1], F32, kind="ExternalInput")
    if with_b2:
        b2_d = nc.dram_tensor("b2", [1, VS], F32, kind="ExternalInput")
    out_d = nc.dram_tensor("out", [KB * P, VS], BF16, kind="ExternalOutput")

    with tile.TileContext(nc) as tc, ExitStack() as ctx:
        const = ctx.enter_context(tc.tile_pool(name="const", bufs=1))
        gpool = ctx.enter_context(tc.tile_pool(name="g8", bufs=3))
        t4pool = ctx.enter_context(tc.tile_pool(name="t4", bufs=2))
        t2pool = ctx.enter_context(tc.tile_pool(name="t2", bufs=2))
        hpool = ctx.enter_context(tc.tile_pool(name="hraw", bufs=4))
        opool = ctx.enter_context(tc.tile_pool(name="out", bufs=31))
        ps_s = ctx.enter_context(tc.tile_pool(name="ps_s", bufs=2, space="PSUM"))
        ps_b = ctx.enter_context(tc.tile_pool(name="ps_b", bufs=3, space="PSUM"))

        # ---- resident tensors -------------------------------------------
        idx_sb = const.tile([P, KB * J], I32, tag="idx")
        nc.sync.dma_start(idx_sb[:], idx_d[:])
        ident = const.tile([P, P], F32, tag="ident")
        make_identity(nc, ident[:])
        b1t = [const.tile([P, 1], F32, tag=f"b1{h}", name=f"b1{h}")
               for h in (0, 1)]
        for h in (0, 1):
            nc.sync.dma_start(b1t[h][:], b1_d[h])
        # resident W2 half, loaded in column segments.  Only the first few
        # segments are issued up front: the DMA engines round-robin across
        # ALL queued transfers, so queueing everything at once makes every
        # segment finish late; later segments are issued from inside block
        # 0's tile loop so delivery stays just ahead of consumption.
        w2r = [const.tile([P, VS], BF16, tag=f"w2r{h}", name=f"w2r{h}")
               for h in (0, 1)]
        wsegs = [(s0, min(WSEG, VS - s0)) for s0 in range(0, VS, WSEG)]

        def load_wseg(s):
            s0, sw = wsegs[s]
            for h in (0, 1):
                nc.sync.dma_start(w2r[h][:, s0:s0 + sw],
                                  w2t_d[h * P:(h + 1) * P, s0:s0 + sw])

        WLEAD = 3
        for s in range(min(WLEAD, len(wsegs))):
            load_wseg(s)
        if with_b2:
            b2bc = const.tile([P, VS], BF16, tag="b2bc")
            nc.sync.dma_start(b2bc[:], b2_d[:].to_broadcast([P, VS]))
        # resident hT tiles for the core's KB blocks
        hts = [[const.tile([P, P], BF16, tag=f"ht{k}{h}", name=f"ht{k}{h}")
                for h in (0, 1)] for k in range(KB)]

        # ---- layer 1 ----------------------------------------------------
        def gather_block(k):
            # gathers only on gpsimd (its elementwise ops are ~2.5x slower
            # than DVE, so the tree does NOT live here)
            g8 = gpool.tile([P, J, D], BF16, tag="g8", name="g8")
            for j in range(J):
                nc.gpsimd.indirect_dma_start(
                    out=g8[:, j, :],
                    out_offset=None,
                    in_=w1t_d[:],
                    in_offset=bass.IndirectOffsetOnAxis(
                        ap=idx_sb[:, k * J + j:k * J + j + 1], axis=0),
                )
            return g8

        def finish_block(k, g8):
            # 3-level DVE tree sum, PE transpose, Scalar relu(x+b1) -> bf16.
            # Emitted mid-stream of the previous block, where the DVE queue
            # holds at most a couple of evictions, so the whole chain
            # completes within a few tiles.
            t4 = t4pool.tile([P, 4, D], BF16, tag="t4", name="t4")
            nc.vector.tensor_add(t4[:], g8[:, 0:4, :], g8[:, 4:8, :])
            t2 = t2pool.tile([P, 2, D], BF16, tag="t2", name="t2")
            nc.vector.tensor_add(t2[:], t4[:, 0:2, :], t4[:, 2:4, :])
            h_raw = hpool.tile([P, D], F32, tag="hraw", name="h_raw")
            nc.vector.tensor_add(h_raw[:], t2[:, 0, :], t2[:, 1, :])
            for h in (0, 1):
                pt = ps_s.tile([P, 512], F32, tag="ps", name="pt")
                nc.tensor.transpose(pt[:, :P], h_raw[:, h * P:(h + 1) * P],
                                    ident[:])
                nc.scalar.activation(hts[k][h][:], pt[:, :P],
                                     mybir.ActivationFunctionType.Relu,
                                     bias=b1t[h][:], scale=1.0)

        g8s = [gather_block(k) for k in range(KB)]
        finish_block(0, g8s[0])

        # ---- layer 2: stream over resident W2, 4 blocks -----------------
        vsub = [(v0, min(VT, VS - v0)) for v0 in range(0, VS, VT)]
        ei = 0
        deferred = []  # block 0's output DMAs, issued during block 1
        for k in range(KB):
            for ti, (v0, vw) in enumerate(vsub):
                # flush one deferred block-0 output write per tile: during
                # block 0 the sync queue issues no output DMAs at all, so
                # the resident-W2 segment loads get the full DMA bandwidth
                # instead of contending with the output stream.
                if k >= 1 and deferred:
                    dst, src = deferred.pop(0)
                    nc.sync.dma_start(dst, src)
                # trickle the remaining W2 segments during block 0, three
                # tiles (~one segment's worth of matmuls) apart
                if k == 0 and ti % 3 == 0:
                    s = WLEAD + ti // 3
                    if s < len(wsegs):
                        load_wseg(s)
                # emit the next block's tree/transpose/relu late in this
                # block's matmul stream: by then its gathers are long done,
                # so the chain completes in a few tiles and the PE
                # transposes slot between matmul batches without
                # head-of-line blocking ready matmuls behind them.
                if ti == 16 and k + 1 < KB:
                    finish_block(k + 1, g8s[k + 1])
                po = ps_b.tile([P, VT], F32, tag="po", name="po")
                for sub in range(0, vw, 512):
                    sw = min(512, vw - sub)
                    for h in (0, 1):
                        nc.tensor.matmul(
                            po[:, sub:sub + sw],
                            lhsT=hts[k][h][:],
                            rhs=w2r[h][:, v0 + sub:v0 + sub + sw],
                            start=(h == 0),
                            stop=(h == 1))
                ot = opool.tile([P, VT], BF16, tag="ot", name="ot")
                if with_b2:
                    nc.vector.tensor_add(ot[:, :vw], po[:, :vw],
                                         b2bc[:, v0:v0 + vw])
                elif ei % 2 == 1:
                    nc.scalar.copy(ot[:, :vw], po[:, :vw])
                else:
                    nc.vector.tensor_copy(ot[:, :vw], po[:, :vw])
                ei += 1
                if k == 0:
                    deferred.append(
                        (out_d[0:P, v0:v0 + vw], ot[:, :vw]))
                else:
                    nc.sync.dma_start(
                        out_d[k * P:(k + 1) * P, v0:v0 + vw], ot[:, :vw])
        assert not deferred

    nc.compile()
    _CACHE[key] = nc
    return nc


def _host_prep(inputs, W1, b1, W2, b2):
    x = np.asarray(inputs)
    assert x.shape == (N, J) and x.dtype == np.int32

    # duplicate mask: scatter-SET semantics -> only first occurrence counts;
    # duplicates are redirected to the all-zero row V of the augmented W1T.
    dup = np.zeros((N, J), dtype=bool)
    for j in range(1, J):
        dup[:, j] = (x[:, :j] == x[:, j:j + 1]).any(axis=1)
    xd = np.where(dup, V, x).astype(np.int32)

    # idxq[q][p, k*J + j] = xd[(4q+k)*128 + p, j]   (batch quarter q)
    idxq = np.ascontiguousarray(
        xd.reshape(C // GV, KB, P, J).transpose(0, 2, 1, 3)
        .reshape(C // GV, P, KB * J))

    w1 = np.asarray(W1, dtype=np.float32)
    w1t = np.concatenate([w1.T * (1.0 / J), np.zeros((1, D), np.float32)],
                         axis=0)
    w1t = np.ascontiguousarray(w1t).astype(ml_dtypes.bfloat16)   # [V+1, D]

    w2t = np.ascontiguousarray(np.asarray(W2, dtype=np.float32).T)  # [D, V]
    w2t = w2t.astype(ml_dtypes.bfloat16)

    b1r = np.ascontiguousarray(np.asarray(b1, dtype=np.float32).reshape(2, P, 1))
    b2f = np.asarray(b2, dtype=np.float32)
    with_b2 = bool(np.any(b2f))

    in_maps = []
    for c in range(C):
        g, q = c // (C // GV), c % (C // GV)
        sl = slice(g * VS, (g + 1) * VS)
        m = {
            "idx": idxq[q],
            "w1t": w1t,
            "w2t": np.ascontiguousarray(w2t[:, sl]),
            "b1": b1r,
        }
        if with_b2:
            m["b2"] = np.ascontiguousarray(b2f[sl].reshape(1, VS))
        in_maps.append(m)
    return in_maps, with_b2


def run(inputs, W1, b1, W2, b2, trace=False):
    from concourse.bass_utils import run_bass_kernel_spmd

    in_maps, with_b2 = _host_prep(inputs, W1, b1, W2, b2)
    nc = _build(with_b2)
    res = run_bass_kernel_spmd(nc, in_maps, core_ids=list(range(C)), trace=trace)
    out = np.empty((N, V), dtype=ml_dtypes.bfloat16)
    for c in range(C):
        g, q = c // (C // GV), c % (C // GV)
        out[q * KB * P:(q + 1) * KB * P, g * VS:(g + 1) * VS] = \
            np.asarray(res.results[c]["out"])
    return out.astype(np.float32), res


def kernel(inputs, W1, b1, W2, b2):
    out, _ = run(inputs, W1, b1, W2, b2, trace=False)
    return out
